# revision 24
# baseline (speedup 1.0000x reference)
"""BioMech feature extraction on Trainium2: 8 NeuronCores, pure data-parallel SPMD.

Self-contained: takes full inputs foot/shank/thigh [8192, 12, 256] fp32,
returns [8192, 44] fp32 feature matrix matching the reference stack order.

Strategy per core (1024 samples, 8 blocks of 128 partitions):
  - Host packs the 22 used channels -> X [B, 22, 256] (foot z+gyro, shank
    z+gyro, thigh gyro), one DMA per 128-sample block.
  - FFT features via a single merged PE matmul with DFT weights
    W = [C(k=0..128) | S(k=1..128) | C*sqrt(k) | S*sqrt(k)] (512 cols,
    exactly one PSUM bank), after a PE transpose of the foot-z channels.
    Sum x^2 comes from Parseval instead of a separate pass.
  - Sums/variances via DVE bn_stats (channel-paired); peaks via fused
    multi-channel abs-max reductions; counts via tensor_scalar
    cache-reduce; ACT Square/Abs/Copy with sum-accumulators for moments.
  - Elementwise products (zcr product, vib diff, c^3, centering) on GpSimd.
  - Per-sample scalar math batched over all blocks at the end.
"""

import contextlib

import numpy as np

import concourse.bacc as bacc
import concourse.tile as tile
import concourse.mybir as mybir
from concourse.bass_utils import run_bass_kernel_spmd

F32 = mybir.dt.float32
AF = mybir.ActivationFunctionType
ALU = mybir.AluOpType
AX = mybir.AxisListType

N_CORES = 8
B_FULL = 8192
T = 256
P = 128
BC = B_FULL // N_CORES          # 1024 samples per core
NBLK = BC // P                  # 8 blocks
NCH = 22
NBINS = 129
HF_BIN = 60
NW = 512                        # merged DFT weight columns
EPS = 1e-6

CH_FOOT = [2, 3, 4, 5, 8, 9, 10, 11]
CH_SHANK = [2, 3, 4, 5, 8, 9, 10, 11]
CH_THIGH = [3, 4, 5, 9, 10, 11]

FZ = (0, 4)          # packed idx of foot z lt/rt
SZ = (8, 12)         # packed idx of shank z lt/rt


def build_consts():
    t = np.arange(T, dtype=np.float64)
    k = np.arange(NBINS, dtype=np.float64)
    ang = 2.0 * np.pi * np.outer(t, k) / T
    C = np.cos(ang)                      # [256, 129] k=0..128
    S = np.sin(ang)
    sk = np.sqrt(k)
    # merged: [C(0..128) | S(1..128) | C*sqrt(k)(1..128) | S*sqrt(k)(1..127)]
    w = np.concatenate([C, S[:, 1:129], (C * sk)[:, 1:129],
                        (S * sk)[:, 1:128]], axis=1)       # [256, 512]
    assert w.shape[1] == NW
    w = np.ascontiguousarray(w.reshape(2, P, NW), dtype=np.float32)
    wh = np.zeros((2, P, 2), dtype=np.float32)
    wh[0, :, 0] = 1.0   # chunk 0 (t<128) -> first-half abs sum
    wh[1, :, 1] = 1.0   # chunk 1 (t>=128) -> second-half abs sum
    ident = np.eye(P, dtype=np.float32)
    return {"w": w, "wh": wh, "ident": ident}


def build_nc():
    nc = bacc.Bacc("TRN2", target_bir_lowering=False, debug=False,
                   num_devices=N_CORES)
    x_d = nc.dram_tensor("x", [BC, NCH, T], F32, kind="ExternalInput")
    w_d = nc.dram_tensor("w", [2, P, NW], mybir.dt.float32r,
                         kind="ExternalInput")
    wh_d = nc.dram_tensor("wh", [2, P, 2], F32, kind="ExternalInput")
    id_d = nc.dram_tensor("ident", [P, P], F32, kind="ExternalInput")
    out_d = nc.dram_tensor("out", [BC, 44], F32, kind="ExternalOutput")

    with tile.TileContext(nc) as tc:
        _body(tc, x_d, w_d, wh_d, id_d, out_d)
    nc.compile()
    return nc


def _body(tc, x_d, w_d, wh_d, id_d, out_d):
    nc = tc.nc
    NS = NBLK * 2        # (block, side) stat slots
    NG = NBLK * 6        # (block, group) slots

    ctx = contextlib.ExitStack()
    with ctx:
        pers = ctx.enter_context(tc.tile_pool(name="pers", bufs=1))
        p_in = ctx.enter_context(tc.tile_pool(name="xin", bufs=3))
        p_psxt = ctx.enter_context(tc.tile_pool(name="psxt", bufs=2, space="PSUM"))
        p_psmm = ctx.enter_context(tc.tile_pool(name="psmm", bufs=3, space="PSUM"))
        p_psh = ctx.enter_context(tc.tile_pool(name="psh", bufs=2, space="PSUM"))
        p_xt = ctx.enter_context(tc.tile_pool(name="xt", bufs=2))
        p_p2 = ctx.enter_context(tc.tile_pool(name="p2", bufs=2))
        p_scr = ctx.enter_context(tc.tile_pool(name="scr", bufs=3))
        p_junk = ctx.enter_context(tc.tile_pool(name="junk", bufs=6))
        p_small = ctx.enter_context(tc.tile_pool(name="small", bufs=3))
        fin = ctx.enter_context(tc.tile_pool(name="fin", bufs=1))

        # ---- constants to SBUF ----
        w_sb = pers.tile([P, 2, NW], mybir.dt.float32r, tag="w", name="w_sb")
        wh_sb = pers.tile([P, 2, 2], F32, tag="wh", name="wh_sb")
        id_sb = pers.tile([P, P], F32, tag="ident", name="id_sb")
        nc.sync.dma_start(out=w_sb[:], in_=w_d.ap().rearrange("j p n -> p j n"))
        nc.sync.dma_start(out=wh_sb[:], in_=wh_d.ap().rearrange("j p n -> p j n"))
        nc.sync.dma_start(out=id_sb[:], in_=id_d.ap())

        def stat(tag, n=NS):
            return pers.tile([P, n], F32, tag=tag, name=tag)

        tot_s = stat("tot")     # sum power (129 bins)
        scn_s = stat("scn")     # sum k*power
        hf_s = stat("hf")       # sum power k>=60 (cos part)
        muN_s = stat("muN", 2 * NS)  # [mean fz, alt-sum/256] per (b, side)
        m3_s = stat("m3")       # sum (fz-mu)^3
        m4_s = stat("m4")       # sum (fz-mu)^4
        imp_s = stat("imp")     # count |fz| >= 0.3 pk
        zc_s = stat("zc")       # zero crossing count
        vib_s = stat("vib")     # sum |diff sz|
        habs_s = stat("habs", 2 * NS)          # [b, side, half]
        bnS_s = pers.tile([P, NS, 6], F32, tag="bnS", name="bnS")
        bnG_s = pers.tile([P, NG * 3, 6], F32, tag="bnG", name="bnG")
        out_t = pers.tile([P, NBLK, 44], F32, tag="out", name="out_t")

        x_ap = x_d.ap()

        for b in range(NBLK):
            Xb = p_in.tile([P, NCH, T], F32, tag="xb", name="Xb")
            nc.sync.dma_start(out=Xb[:], in_=x_ap[b * P:(b + 1) * P, :, :])

            # fz peaks for both sides in one fused reduce [p, 2, 256]
            # (emitted before the channel loop: thr below reads pkF_s)
            fzv = Xb[:, 0:8, :].rearrange("p (s c) t -> p s c t", s=2)[:, :, 0, :]
            nc.vector.tensor_reduce(out_t[:, b, 0:2], fzv, axis=AX.X,
                                    op=ALU.max, apply_absolute_value=True)

            # ---------- foot z channels ----------
            for side, ch in enumerate(FZ):
                x = Xb[:, ch, :]
                i2 = 2 * b + side
                xt_ps = p_psxt.tile([P, 2, P], F32, tag="xtps", name="xt_ps")
                nc.tensor.transpose(xt_ps[:, 0, :], x[:, 0:P], id_sb[:])
                nc.tensor.transpose(xt_ps[:, 1, :], x[:, P:T], id_sb[:])
                XT = p_xt.tile([P, 2, P], mybir.dt.float32r, tag="xt",
                               name="XT")
                nc.scalar.copy(XT[:], xt_ps[:])
                AXT = p_xt.tile([P, 2, P], F32, tag="axt", name="AXT")
                nc.scalar.activation(AXT[:], xt_ps[:], AF.Abs)

                psF = p_psmm.tile([P, NW], F32, tag="psF", name="psF")
                psH = p_psh.tile([P, 2], F32, tag="psH", name="psH")
                for j in (0, 1):
                    nc.tensor.matmul(psF[:], XT[:, j, :], w_sb[:, j, :],
                                     start=(j == 0), stop=(j == 1))
                for j in (0, 1):
                    nc.tensor.matmul(psH[:], AXT[:, j, :], wh_sb[:, j, :],
                                     start=(j == 0), stop=(j == 1))

                P2 = p_p2.tile([P, 257], F32, tag="p2", name="P2")
                nc.scalar.activation(P2[:], psF[:, 0:257], AF.Square,
                                     accum_out=tot_s[:, i2:i2 + 1])
                jB = p_junk.tile([P, 255], F32, tag="junkb", name="jB")
                nc.scalar.activation(jB[:], psF[:, 257:NW], AF.Square,
                                     accum_out=scn_s[:, i2:i2 + 1])
                # hf sum over {cos k>=60} u {sin k>=60}: two 69-wide ranges
                # 128 apart -> one strided reduce
                import concourse.bass as _bass
                p2b = P2[:]
                hfap = _bass.AP(tensor=p2b.tensor,
                                offset=p2b.offset + HF_BIN,
                                ap=[p2b.ap[0], [P, 2], [1, NBINS - HF_BIN]])
                nc.vector.reduce_sum(hf_s[:, i2:i2 + 1], hfap, axis=AX.XY)
                nc.scalar.copy(habs_s[:, 2 * i2:2 * i2 + 2], psH[:])
                # cols 0 and 128 of psF = sum(x) and alternating sum -> both
                # scaled by 1/256: mu and aN (Parseval: p0 = (256 mu)^2 etc.)
                nc.scalar.activation(muN_s[:, 2 * i2:2 * i2 + 2],
                                     psF[:, 0:NBINS:P], AF.Copy,
                                     scale=1.0 / T)

                ABSX = p_scr.tile([P, T], F32, tag="absx", name="ABSX")
                nc.scalar.activation(ABSX[:], x, AF.Abs)
                thr = p_small.tile([P, 1], F32, tag="thr", name="thr")
                nc.vector.tensor_scalar(thr[:], out_t[:, b, side:side + 1],
                                        0.3, None, op0=ALU.mult)
                j1 = p_junk.tile([P, T], F32, tag="junk", name="j1")
                nc.vector.tensor_scalar(j1[:], ABSX[:], thr[:], None,
                                        op0=ALU.is_ge, op1=ALU.add,
                                        accum_out=imp_s[:, i2:i2 + 1])

                C = p_scr.tile([P, T], F32, tag="c", name="C")
                nc.vector.tensor_scalar(C[:], x, muN_s[:, 2 * i2:2 * i2 + 1],
                                        None, op0=ALU.subtract)
                C2 = p_scr.tile([P, T], F32, tag="c2", name="C2")
                nc.scalar.activation(C2[:], C[:], AF.Square)
                j3 = p_junk.tile([P, T], F32, tag="junk", name="j3")
                nc.scalar.activation(j3[:], C2[:], AF.Square,
                                     accum_out=m4_s[:, i2:i2 + 1])
                C3 = p_scr.tile([P, T], F32, tag="c3", name="C3")
                nc.gpsimd.tensor_tensor(C3[:], C2[:], C[:], op=ALU.mult)
                j2 = p_junk.tile([P, T], F32, tag="junk", name="j2")
                nc.scalar.activation(j2[:], C3[:], AF.Copy,
                                     accum_out=m3_s[:, i2:i2 + 1])

                PR = p_scr.tile([P, T - 1], F32, tag="pr", name="PR")
                nc.gpsimd.tensor_tensor(PR[:], x[:, 1:T], x[:, 0:T - 1],
                                        op=ALU.mult)
                j4 = p_junk.tile([P, T], F32, tag="junk", name="j4")
                nc.vector.tensor_scalar(j4[:, 0:T - 1], PR[:], 0.0, None,
                                        op0=ALU.is_lt, op1=ALU.add,
                                        accum_out=zc_s[:, i2:i2 + 1])

            # ---------- shank z channels ----------
            szv = Xb[:, 8:16, :].rearrange("p (s c) t -> p s c t", s=2)[:, :, 0, :]
            nc.vector.tensor_reduce(out_t[:, b, 2:4], szv, axis=AX.X,
                                    op=ALU.max, apply_absolute_value=True)
            for side, ch in enumerate(SZ):
                nc.vector.bn_stats(bnS_s[:, 2 * b + side, :], Xb[:, ch, :])
                y = Xb[:, ch, :]
                i2 = 2 * b + side
                D = p_scr.tile([P, T - 1], F32, tag="d", name="D")
                nc.gpsimd.tensor_tensor(D[:], y[:, 1:T], y[:, 0:T - 1],
                                        op=ALU.subtract)
                j6 = p_junk.tile([P, T], F32, tag="junk", name="j6")
                nc.scalar.activation(j6[:, 0:T - 1], D[:], AF.Abs,
                                     accum_out=vib_s[:, i2:i2 + 1])

            # ---------- gyro groups ----------
            # peaks: foot+shank groups (start ch 1,5,9,13) in one reduce,
            # thigh groups (16..21) in another
            gfs = Xb[:, 1:17, :].rearrange("p (g c) t -> p g c t", g=4)[:, :, 0:3, :]
            vqb = out_t[:, b, 20:32].rearrange("p (j q) -> p j q", q=4)
            nc.vector.tensor_reduce(vqb[:, 0:2, 2:4], gfs, axis=AX.XY,
                                    op=ALU.max, apply_absolute_value=True)
            gth = Xb[:, 16:22, :].rearrange("p (g c) t -> p g c t", g=2)
            nc.vector.tensor_reduce(vqb[:, 2, 2:4], gth, axis=AX.XY,
                                    op=ALU.max, apply_absolute_value=True)
            for gi, c0 in enumerate((1, 5, 9, 13, 16, 19)):
                ig = 6 * b + gi
                for cc in range(3):
                    nc.vector.bn_stats(bnG_s[:, ig * 3 + cc, :],
                                       Xb[:, c0 + cc, :])

        # ================= final batched scalar phase =================
        def v2(tbl):
            return tbl[:].rearrange("p (b s) -> p b s", s=2)

        def ft(tag, shape=(P, NBLK, 2)):
            return fin.tile(list(shape), F32, tag=tag, name=tag)

        V = nc.vector
        SC = nc.scalar

        # ratio = log1p(f_pk / (s_pk + 1e-4))  [Ln later]
        r_spk = ft("r_spk")
        V.tensor_scalar(r_spk[:], out_t[:, :, 2:4], 1e-4, None, op0=ALU.add)
        V.reciprocal(r_spk[:], r_spk[:])
        ratio_arg = ft("ratio_arg")
        V.tensor_mul(ratio_arg[:], out_t[:, :, 0:2], r_spk[:])

        # total power reciprocal (shared by hf and sc)
        r_tot = ft("r_tot")
        V.tensor_scalar(r_tot[:], v2(tot_s), EPS, None, op0=ALU.add)
        V.reciprocal(r_tot[:], r_tot[:])
        V.tensor_mul(out_t[:, :, 6:8], v2(hf_s), r_tot[:])
        V.scalar_tensor_tensor(out_t[:, :, 16:18], v2(scn_s), 1.0 / NBINS,
                               r_tot[:], op0=ALU.mult, op1=ALU.mult)

        # dur, vib, zcr scaled counts
        V.tensor_scalar(out_t[:, :, 18:20], v2(imp_s), 1.0 / T, None, op0=ALU.mult)
        V.tensor_scalar(out_t[:, :, 12:14], v2(vib_s), 1.0 / (T - 1), None, op0=ALU.mult)
        V.tensor_scalar(out_t[:, :, 42:44], v2(zc_s), 1.0 / (T - 1), None, op0=ALU.mult)

        # decay = h0 / (h1 + 128e-6)
        hv = habs_s[:].rearrange("p (b s h) -> p b s h", s=2, h=2)
        dden = ft("dden")
        V.tensor_scalar(dden[:], hv[:, :, :, 1], (T // 2) * EPS, None, op0=ALU.add)
        V.reciprocal(dden[:], dden[:])
        V.tensor_mul(out_t[:, :, 10:12], hv[:, :, :, 0], dden[:])

        # asym_acc = |f_pk - s_pk|
        aa = ft("aa")
        V.tensor_sub(aa[:], out_t[:, :, 0:2], out_t[:, :, 2:4])
        SC.activation(out_t[:, :, 32:34], aa[:], AF.Abs)

        # foot m2 via Parseval:
        # sum x^2 = (2 tot - p0 - pN)/256 with p0 = (256 mu)^2, pN = (256 aN)^2
        #         = tot/128 - 256 (mu^2 + aN^2)
        mv = muN_s[:].rearrange("p (b s h) -> p b s h", s=2, h=2)
        qF = ft("qF")
        V.tensor_mul(qF[:], mv[:, :, :, 0], mv[:, :, :, 0])
        qN = ft("qN")
        V.tensor_mul(qN[:], mv[:, :, :, 1], mv[:, :, :, 1])
        qsum = ft("qsum")
        V.tensor_add(qsum[:], qF[:], qN[:])
        tq = ft("tq")
        V.tensor_scalar(tq[:], v2(tot_s), 1.0 / (T // 2), None, op0=ALU.mult)
        sqF = ft("sqF")           # sum x^2
        V.scalar_tensor_tensor(sqF[:], qsum[:], -float(T), tq[:],
                               op0=ALU.mult, op1=ALU.add)
        m2F = ft("m2F")           # sum (x-mu)^2
        V.scalar_tensor_tensor(m2F[:], qF[:], -float(T), sqF[:],
                               op0=ALU.mult, op1=ALU.add)

        # shank bn-derived: sum y^2 and M2
        meS = bnS_s[:, :, 1].rearrange("p (b s) -> p b s", s=2)
        moS = bnS_s[:, :, 4].rearrange("p (b s) -> p b s", s=2)
        ceS = bnS_s[:, :, 2].rearrange("p (b s) -> p b s", s=2)
        coS = bnS_s[:, :, 5].rearrange("p (b s) -> p b s", s=2)
        sAS = ft("sAS")
        V.tensor_add(sAS[:], ceS, coS)
        uS = ft("uS")
        V.tensor_mul(uS[:], meS, meS)
        vS = ft("vS")
        V.tensor_mul(vS[:], moS, moS)
        wS = ft("wS")
        V.tensor_add(wS[:], uS[:], vS[:])
        sqS = ft("sqS")          # sum y^2
        V.scalar_tensor_tensor(sqS[:], wS[:], float(T // 2), sAS[:],
                               op0=ALU.mult, op1=ALU.add)
        musS = ft("musS")
        V.tensor_add(musS[:], meS, moS)
        qS = ft("qS")
        V.tensor_mul(qS[:], musS[:], musS[:])
        M2S = ft("M2S")          # sum (y - mu)^2
        V.scalar_tensor_tensor(M2S[:], qS[:], -float(T) / 4.0, sqS[:],
                               op0=ALU.mult, op1=ALU.add)

        # var_ratio = log1p(m2F / (M2S + 255e-4))  [Ln later]
        vr = ft("vr")
        V.tensor_scalar(vr[:], M2S[:], (T - 1) * 1e-4, None, op0=ALU.add)
        V.reciprocal(vr[:], vr[:])
        vra = ft("vra")
        V.tensor_mul(vra[:], m2F[:], vr[:])

        # gyro bn-derived per channel -> M2 summed over the 3 channels
        meG = bnG_s[:, :, 1]     # [P, NG*3] strided
        moG = bnG_s[:, :, 4]
        ceG = bnG_s[:, :, 2]
        coG = bnG_s[:, :, 5]
        NG3 = NG * 3
        sAG = ft("sAG", (P, NG3))
        V.tensor_add(sAG[:], ceG, coG)
        uG = ft("uG", (P, NG3))
        V.tensor_mul(uG[:], meG, meG)
        vG = ft("vG", (P, NG3))
        V.tensor_mul(vG[:], moG, moG)
        wG = ft("wG", (P, NG3))
        V.tensor_add(wG[:], uG[:], vG[:])
        sqG = ft("sqG", (P, NG3))
        V.scalar_tensor_tensor(sqG[:], wG[:], float(T // 2), sAG[:],
                               op0=ALU.mult, op1=ALU.add)
        musG = ft("musG", (P, NG3))
        V.tensor_add(musG[:], meG, moG)
        qG = ft("qG", (P, NG3))
        V.tensor_mul(qG[:], musG[:], musG[:])
        M2G = ft("M2G", (P, NG3))
        V.scalar_tensor_tensor(M2G[:], qG[:], -float(T) / 4.0, sqG[:],
                               op0=ALU.mult, op1=ALU.add)
        gM2 = ft("gM2", (P, NG))
        V.reduce_sum(gM2[:], M2G[:].rearrange("p (g c) -> p g c", c=3), axis=AX.X)

        vq = out_t[:, :, 20:32].rearrange("p b (j q) -> p b j q", q=4)

        # ---- Sqrt-set ACT ops ----
        SC.activation(out_t[:, :, 8:10], m2F[:], AF.Sqrt, scale=1.0 / (T - 1))
        rmsF = ft("rmsF")
        SC.activation(rmsF[:], sqF[:], AF.Sqrt, scale=1.0 / T)
        rmsS = ft("rmsS")
        SC.activation(rmsS[:], sqS[:], AF.Sqrt, scale=1.0 / T)

        # kurt/skew (uses std at out[...,8:10])
        sg = ft("sg")
        V.tensor_scalar(sg[:], out_t[:, :, 8:10], 1e-6, None, op0=ALU.max)
        vv = ft("vv")
        V.tensor_mul(vv[:], sg[:], sg[:])
        v4 = ft("v4")
        V.tensor_mul(v4[:], vv[:], vv[:])
        V.reciprocal(v4[:], v4[:])
        kr = ft("kr")
        V.scalar_tensor_tensor(kr[:], v2(m4_s), 1.0 / T, v4[:],
                               op0=ALU.mult, op1=ALU.mult)
        V.tensor_scalar(out_t[:, :, 38:40], kr[:], 30.0, -10.0,
                        op0=ALU.min, op1=ALU.max)
        v3 = ft("v3")
        V.tensor_mul(v3[:], vv[:], sg[:])
        V.reciprocal(v3[:], v3[:])
        sk = ft("sk")
        V.scalar_tensor_tensor(sk[:], v2(m3_s), 1.0 / T, v3[:],
                               op0=ALU.mult, op1=ALU.mult)
        V.tensor_scalar(out_t[:, :, 40:42], sk[:], 10.0, -10.0,
                        op0=ALU.min, op1=ALU.max)

        # trans arg = rmsS / (rmsF + 1e-6)
        rdn = ft("rdn")
        V.tensor_scalar(rdn[:], rmsF[:], EPS, None, op0=ALU.add)
        V.reciprocal(rdn[:], rdn[:])
        targ = ft("targ")
        V.tensor_mul(targ[:], rmsS[:], rdn[:])

        # ---- Ln-set ACT ops (log1p via bias=1) ----
        SC.activation(out_t[:, :, 4:6], ratio_arg[:], AF.Ln, bias=1.0)
        SC.activation(out_t[:, :, 14:16], vra[:], AF.Ln, bias=1.0)
        SC.activation(out_t[:, :, 36:38], targ[:], AF.Ln, bias=1.0)
        SC.activation(vq[:, :, :, 0:2],
                      gM2[:].rearrange("p (b j l) -> p b j l", j=3, l=2),
                      AF.Ln, scale=1.0 / (T - 1), bias=1.0)

        # asym_gy = |fg_var - sg_var| (after log1p)
        ag = ft("ag")
        V.tensor_sub(ag[:], out_t[:, :, 20:22], out_t[:, :, 24:26])
        SC.activation(out_t[:, :, 34:36], ag[:], AF.Abs)

        # ---- store ----
        nc.sync.dma_start(out=out_d.ap().rearrange("(b p) f -> p b f", p=P),
                          in_=out_t[:])


_NC_CACHE = None
_CONSTS = None


def _get_nc():
    global _NC_CACHE, _CONSTS
    if _NC_CACHE is None:
        _NC_CACHE = build_nc()
    if _CONSTS is None:
        _CONSTS = build_consts()
    return _NC_CACHE, _CONSTS


def run(foot, shank, thigh, **kw):
    foot = np.asarray(foot, dtype=np.float32)
    shank = np.asarray(shank, dtype=np.float32)
    thigh = np.asarray(thigh, dtype=np.float32)
    X = np.empty((B_FULL, NCH, T), dtype=np.float32)
    X[:, 0:8] = foot[:, CH_FOOT]
    X[:, 8:16] = shank[:, CH_SHANK]
    X[:, 16:22] = thigh[:, CH_THIGH]

    nc, consts = _get_nc()
    in_maps = []
    for i in range(N_CORES):
        in_maps.append({
            "x": np.ascontiguousarray(X[i * BC:(i + 1) * BC]),
            "w": consts["w"], "wh": consts["wh"], "ident": consts["ident"],
        })
    return run_bass_kernel_spmd(nc, in_maps, core_ids=list(range(N_CORES)),
                                **kw)


def kernel(foot, shank, thigh):
    res = run(foot, shank, thigh)
    return np.concatenate([res.results[i]["out"] for i in range(N_CORES)],
                          axis=0)


# revision 25
# speedup vs baseline: 1.1556x; 1.1556x over previous
"""BioMech feature extraction on Trainium2: 8 NeuronCores, pure data-parallel SPMD.

Self-contained: takes full inputs foot/shank/thigh [8192, 12, 256] fp32,
returns [8192, 44] fp32 feature matrix matching the reference stack order.

Strategy per core (1024 samples, 8 blocks of 128 partitions):
  - Host packs the 22 used channels -> X [B, 22, 256] (foot z+gyro, shank
    z+gyro, thigh gyro), one DMA per 128-sample block.
  - FFT features via a single merged PE matmul with DFT weights
    W = [C(k=0..128) | S(k=1..128) | C*sqrt(k) | S*sqrt(k)] (512 cols,
    exactly one PSUM bank), after a PE transpose of the foot-z channels.
    Sum x^2 comes from Parseval instead of a separate pass.
  - Sums/variances via DVE bn_stats (channel-paired); peaks via fused
    multi-channel abs-max reductions; counts via tensor_scalar
    cache-reduce; ACT Square/Abs/Copy with sum-accumulators for moments.
  - Elementwise products (zcr product, vib diff, c^3, centering) on GpSimd.
  - Per-sample scalar math batched over all blocks at the end.
"""

import contextlib

import numpy as np

import concourse.bacc as bacc
import concourse.tile as tile
import concourse.mybir as mybir
from concourse.bass_utils import run_bass_kernel_spmd

F32 = mybir.dt.float32
AF = mybir.ActivationFunctionType
ALU = mybir.AluOpType
AX = mybir.AxisListType

N_CORES = 8
B_FULL = 8192
T = 256
P = 128
BC = B_FULL // N_CORES          # 1024 samples per core
NBLK = BC // P                  # 8 blocks
NCH = 22
NBINS = 129
HF_BIN = 60
NW = 512                        # merged DFT weight columns
EPS = 1e-6

CH_FOOT = [2, 3, 4, 5, 8, 9, 10, 11]
CH_SHANK = [2, 3, 4, 5, 8, 9, 10, 11]
CH_THIGH = [3, 4, 5, 9, 10, 11]

FZ = (0, 4)          # packed idx of foot z lt/rt
SZ = (8, 12)         # packed idx of shank z lt/rt


def build_consts():
    t = np.arange(T, dtype=np.float64)
    k = np.arange(NBINS, dtype=np.float64)
    ang = 2.0 * np.pi * np.outer(t, k) / T
    C = np.cos(ang)                      # [256, 129] k=0..128
    S = np.sin(ang)
    sk = np.sqrt(k)
    # merged: [C(0..128) | S(1..128) | C*sqrt(k)(1..128) | S*sqrt(k)(1..127)]
    w = np.concatenate([C, S[:, 1:129], (C * sk)[:, 1:129],
                        (S * sk)[:, 1:128]], axis=1)       # [256, 512]
    assert w.shape[1] == NW
    w = np.ascontiguousarray(w.reshape(2, P, NW), dtype=np.float32)
    wh = np.zeros((2, P, 2), dtype=np.float32)
    wh[0, :, 0] = 1.0   # chunk 0 (t<128) -> first-half abs sum
    wh[1, :, 1] = 1.0   # chunk 1 (t>=128) -> second-half abs sum
    ident = np.eye(P, dtype=np.float32)
    return {"w": w, "wh": wh, "ident": ident}


def build_nc():
    nc = bacc.Bacc("TRN2", target_bir_lowering=False, debug=False,
                   num_devices=N_CORES)
    x_d = nc.dram_tensor("x", [BC, NCH, T], F32, kind="ExternalInput")
    w_d = nc.dram_tensor("w", [2, P, NW], mybir.dt.float32r,
                         kind="ExternalInput")
    wh_d = nc.dram_tensor("wh", [2, P, 2], F32, kind="ExternalInput")
    id_d = nc.dram_tensor("ident", [P, P], F32, kind="ExternalInput")
    out_d = nc.dram_tensor("out", [BC, 44], F32, kind="ExternalOutput")

    with tile.TileContext(nc) as tc:
        _body(tc, x_d, w_d, wh_d, id_d, out_d)
    nc.compile()
    return nc


def _body(tc, x_d, w_d, wh_d, id_d, out_d):
    nc = tc.nc
    NS = NBLK * 2        # (block, side) stat slots
    NG = NBLK * 6        # (block, group) slots

    ctx = contextlib.ExitStack()
    with ctx:
        pers = ctx.enter_context(tc.tile_pool(name="pers", bufs=1))
        p_in = ctx.enter_context(tc.tile_pool(name="xin", bufs=3))
        p_psxt = ctx.enter_context(tc.tile_pool(name="psxt", bufs=2, space="PSUM"))
        p_psmm = ctx.enter_context(tc.tile_pool(name="psmm", bufs=3, space="PSUM"))
        p_psh = ctx.enter_context(tc.tile_pool(name="psh", bufs=2, space="PSUM"))
        p_xt = ctx.enter_context(tc.tile_pool(name="xt", bufs=2))
        p_p2 = ctx.enter_context(tc.tile_pool(name="p2", bufs=2))
        p_scr = ctx.enter_context(tc.tile_pool(name="scr", bufs=3))
        p_junk = ctx.enter_context(tc.tile_pool(name="junk", bufs=6))
        p_small = ctx.enter_context(tc.tile_pool(name="small", bufs=3))
        fin = ctx.enter_context(tc.tile_pool(name="fin", bufs=1))

        # ---- constants to SBUF ----
        w_sb = pers.tile([P, 2, NW], mybir.dt.float32r, tag="w", name="w_sb")
        wh_sb = pers.tile([P, 2, 2], F32, tag="wh", name="wh_sb")
        id_sb = pers.tile([P, P], F32, tag="ident", name="id_sb")
        nc.sync.dma_start(out=w_sb[:], in_=w_d.ap().rearrange("j p n -> p j n"))
        nc.sync.dma_start(out=wh_sb[:], in_=wh_d.ap().rearrange("j p n -> p j n"))
        nc.sync.dma_start(out=id_sb[:], in_=id_d.ap())

        def stat(tag, n=NS):
            return pers.tile([P, n], F32, tag=tag, name=tag)

        tot_s = stat("tot")     # sum power (129 bins)
        scn_s = stat("scn")     # sum k*power
        hf_s = stat("hf")       # sum power k>=60 (cos part)
        muN_s = stat("muN", 2 * NS)  # [mean fz, alt-sum/256] per (b, side)
        m3_s = stat("m3")       # sum (fz-mu)^3
        m4_s = stat("m4")       # sum (fz-mu)^4
        imp_s = stat("imp")     # count |fz| >= 0.3 pk
        zc_s = stat("zc")       # zero crossing count
        vib_s = stat("vib")     # sum |diff sz|
        habs_s = stat("habs", 2 * NS)          # [b, side, half]
        bnS_s = pers.tile([P, NS, 6], F32, tag="bnS", name="bnS")
        bnG_s = pers.tile([P, NG * 3, 6], F32, tag="bnG", name="bnG")
        out_t = pers.tile([P, NBLK, 44], F32, tag="out", name="out_t")

        x_ap = x_d.ap()

        for b in range(NBLK):
            Xb = p_in.tile([P, NCH, T], F32, tag="xb", name="Xb")
            nc.sync.dma_start(out=Xb[:], in_=x_ap[b * P:(b + 1) * P, :, :])

            # fz peaks for both sides in one fused reduce [p, 2, 256]
            # (emitted before the channel loop: thr below reads pkF_s)
            fzv = Xb[:, 0:8, :].rearrange("p (s c) t -> p s c t", s=2)[:, :, 0, :]
            nc.vector.tensor_reduce(out_t[:, b, 0:2], fzv, axis=AX.X,
                                    op=ALU.max, apply_absolute_value=True)

            # ---------- foot z channels ----------
            for side, ch in enumerate(FZ):
                x = Xb[:, ch, :]
                i2 = 2 * b + side
                xt_ps = p_psxt.tile([P, 2, P], F32, tag="xtps", name="xt_ps")
                nc.tensor.transpose(xt_ps[:, 0, :], x[:, 0:P], id_sb[:])
                nc.tensor.transpose(xt_ps[:, 1, :], x[:, P:T], id_sb[:])
                XT = p_xt.tile([P, 2, P], mybir.dt.float32r, tag="xt",
                               name="XT")
                nc.scalar.copy(XT[:], xt_ps[:])
                AXT = p_xt.tile([P, 2, P], F32, tag="axt", name="AXT")
                nc.scalar.activation(AXT[:], xt_ps[:], AF.Abs)

                psF = p_psmm.tile([P, NW], F32, tag="psF", name="psF")
                psH = p_psh.tile([P, 2], F32, tag="psH", name="psH")
                for j in (0, 1):
                    nc.tensor.matmul(psF[:], XT[:, j, :], w_sb[:, j, :],
                                     start=(j == 0), stop=(j == 1))
                for j in (0, 1):
                    nc.tensor.matmul(psH[:], AXT[:, j, :], wh_sb[:, j, :],
                                     start=(j == 0), stop=(j == 1))

                P2 = p_p2.tile([P, 257], F32, tag="p2", name="P2")
                nc.scalar.activation(P2[:], psF[:, 0:257], AF.Square,
                                     accum_out=tot_s[:, i2:i2 + 1])
                jB = p_junk.tile([P, 255], F32, tag="junkb", name="jB")
                nc.scalar.activation(jB[:], psF[:, 257:NW], AF.Square,
                                     accum_out=scn_s[:, i2:i2 + 1])
                # hf sum over {cos k>=60} u {sin k>=60}: two 69-wide ranges
                # 128 apart -> one strided reduce
                import concourse.bass as _bass
                p2b = P2[:]
                hfap = _bass.AP(tensor=p2b.tensor,
                                offset=p2b.offset + HF_BIN,
                                ap=[p2b.ap[0], [P, 2], [1, NBINS - HF_BIN]])
                nc.vector.reduce_sum(hf_s[:, i2:i2 + 1], hfap, axis=AX.XY)
                nc.scalar.copy(habs_s[:, 2 * i2:2 * i2 + 2], psH[:])
                # cols 0 and 128 of psF = sum(x) and alternating sum -> both
                # scaled by 1/256: mu and aN (Parseval: p0 = (256 mu)^2 etc.)
                nc.scalar.activation(muN_s[:, 2 * i2:2 * i2 + 2],
                                     psF[:, 0:NBINS:P], AF.Copy,
                                     scale=-1.0 / T)

                ABSX = p_scr.tile([P, T], F32, tag="absx", name="ABSX")
                nc.scalar.activation(ABSX[:], x, AF.Abs)
                thr = p_small.tile([P, 1], F32, tag="thr", name="thr")
                nc.vector.tensor_scalar(thr[:], out_t[:, b, side:side + 1],
                                        -0.3, None, op0=ALU.mult)
                j1 = p_junk.tile([P, T], F32, tag="junk", name="j1")
                # sum sign(|x| - 0.3pk); count_above = (256 + S)/2
                nc.scalar.activation(j1[:], ABSX[:], AF.Sign, bias=thr[:],
                                     accum_out=imp_s[:, i2:i2 + 1])

                C = p_scr.tile([P, T], F32, tag="c", name="C")
                nc.scalar.activation(C[:], x, AF.Identity,
                                     bias=muN_s[:, 2 * i2:2 * i2 + 1])
                C2 = p_scr.tile([P, T], F32, tag="c2", name="C2")
                nc.scalar.activation(C2[:], C[:], AF.Square)
                j3 = p_junk.tile([P, T], F32, tag="junk", name="j3")
                nc.scalar.activation(j3[:], C2[:], AF.Square,
                                     accum_out=m4_s[:, i2:i2 + 1])
                C3 = p_scr.tile([P, T], F32, tag="c3", name="C3")
                nc.gpsimd.tensor_tensor(C3[:], C2[:], C[:], op=ALU.mult)
                j2 = p_junk.tile([P, T], F32, tag="junk", name="j2")
                nc.scalar.activation(j2[:], C3[:], AF.Copy,
                                     accum_out=m3_s[:, i2:i2 + 1])

                PR = p_scr.tile([P, T - 1], F32, tag="pr", name="PR")
                nc.gpsimd.tensor_tensor(PR[:], x[:, 1:T], x[:, 0:T - 1],
                                        op=ALU.mult)
                j4 = p_junk.tile([P, T], F32, tag="junk", name="j4")
                # sum sign(x_t x_{t+1}); crossings = (255 - S)/2
                nc.scalar.activation(j4[:, 0:T - 1], PR[:], AF.Sign,
                                     accum_out=zc_s[:, i2:i2 + 1])

            # ---------- shank z channels ----------
            szv = Xb[:, 8:16, :].rearrange("p (s c) t -> p s c t", s=2)[:, :, 0, :]
            nc.vector.tensor_reduce(out_t[:, b, 2:4], szv, axis=AX.X,
                                    op=ALU.max, apply_absolute_value=True)
            for side, ch in enumerate(SZ):
                nc.vector.bn_stats(bnS_s[:, 2 * b + side, :], Xb[:, ch, :])
                y = Xb[:, ch, :]
                i2 = 2 * b + side
                D = p_scr.tile([P, T - 1], F32, tag="d", name="D")
                nc.gpsimd.tensor_tensor(D[:], y[:, 1:T], y[:, 0:T - 1],
                                        op=ALU.subtract)
                j6 = p_junk.tile([P, T], F32, tag="junk", name="j6")
                nc.scalar.activation(j6[:, 0:T - 1], D[:], AF.Abs,
                                     accum_out=vib_s[:, i2:i2 + 1])

            # ---------- gyro groups ----------
            # peaks: foot+shank groups (start ch 1,5,9,13) in one reduce,
            # thigh groups (16..21) in another
            gfs = Xb[:, 1:17, :].rearrange("p (g c) t -> p g c t", g=4)[:, :, 0:3, :]
            vqb = out_t[:, b, 20:32].rearrange("p (j q) -> p j q", q=4)
            nc.vector.tensor_reduce(vqb[:, 0:2, 2:4], gfs, axis=AX.XY,
                                    op=ALU.max, apply_absolute_value=True)
            gth = Xb[:, 16:22, :].rearrange("p (g c) t -> p g c t", g=2)
            nc.vector.tensor_reduce(vqb[:, 2, 2:4], gth, axis=AX.XY,
                                    op=ALU.max, apply_absolute_value=True)
            for gi, c0 in enumerate((1, 5, 9, 13, 16, 19)):
                ig = 6 * b + gi
                for cc in range(3):
                    nc.vector.bn_stats(bnG_s[:, ig * 3 + cc, :],
                                       Xb[:, c0 + cc, :])

        # ================= final batched scalar phase =================
        def v2(tbl):
            return tbl[:].rearrange("p (b s) -> p b s", s=2)

        def ft(tag, shape=(P, NBLK, 2)):
            return fin.tile(list(shape), F32, tag=tag, name=tag)

        V = nc.vector
        SC = nc.scalar

        # ratio = log1p(f_pk / (s_pk + 1e-4))  [Ln later]
        r_spk = ft("r_spk")
        V.tensor_scalar(r_spk[:], out_t[:, :, 2:4], 1e-4, None, op0=ALU.add)
        V.reciprocal(r_spk[:], r_spk[:])
        ratio_arg = ft("ratio_arg")
        V.tensor_mul(ratio_arg[:], out_t[:, :, 0:2], r_spk[:])

        # total power reciprocal (shared by hf and sc)
        r_tot = ft("r_tot")
        V.tensor_scalar(r_tot[:], v2(tot_s), EPS, None, op0=ALU.add)
        V.reciprocal(r_tot[:], r_tot[:])
        V.tensor_mul(out_t[:, :, 6:8], v2(hf_s), r_tot[:])
        V.scalar_tensor_tensor(out_t[:, :, 16:18], v2(scn_s), 1.0 / NBINS,
                               r_tot[:], op0=ALU.mult, op1=ALU.mult)

        # dur, vib, zcr scaled counts
        V.tensor_scalar(out_t[:, :, 18:20], v2(imp_s), 1.0 / (2 * T), 0.5,
                        op0=ALU.mult, op1=ALU.add)
        V.tensor_scalar(out_t[:, :, 12:14], v2(vib_s), 1.0 / (T - 1), None, op0=ALU.mult)
        V.tensor_scalar(out_t[:, :, 42:44], v2(zc_s), -1.0 / (2 * (T - 1)),
                        0.5, op0=ALU.mult, op1=ALU.add)

        # decay = h0 / (h1 + 128e-6)
        hv = habs_s[:].rearrange("p (b s h) -> p b s h", s=2, h=2)
        dden = ft("dden")
        V.tensor_scalar(dden[:], hv[:, :, :, 1], (T // 2) * EPS, None, op0=ALU.add)
        V.reciprocal(dden[:], dden[:])
        V.tensor_mul(out_t[:, :, 10:12], hv[:, :, :, 0], dden[:])

        # asym_acc = |f_pk - s_pk|
        aa = ft("aa")
        V.tensor_sub(aa[:], out_t[:, :, 0:2], out_t[:, :, 2:4])
        SC.activation(out_t[:, :, 32:34], aa[:], AF.Abs)

        # foot m2 via Parseval:
        # sum x^2 = (2 tot - p0 - pN)/256 with p0 = (256 mu)^2, pN = (256 aN)^2
        #         = tot/128 - 256 (mu^2 + aN^2)
        mv = muN_s[:].rearrange("p (b s h) -> p b s h", s=2, h=2)
        qF = ft("qF")
        V.tensor_mul(qF[:], mv[:, :, :, 0], mv[:, :, :, 0])
        qN = ft("qN")
        V.tensor_mul(qN[:], mv[:, :, :, 1], mv[:, :, :, 1])
        qsum = ft("qsum")
        V.tensor_add(qsum[:], qF[:], qN[:])
        tq = ft("tq")
        V.tensor_scalar(tq[:], v2(tot_s), 1.0 / (T // 2), None, op0=ALU.mult)
        sqF = ft("sqF")           # sum x^2
        V.scalar_tensor_tensor(sqF[:], qsum[:], -float(T), tq[:],
                               op0=ALU.mult, op1=ALU.add)
        m2F = ft("m2F")           # sum (x-mu)^2
        V.scalar_tensor_tensor(m2F[:], qF[:], -float(T), sqF[:],
                               op0=ALU.mult, op1=ALU.add)

        # shank bn-derived: sum y^2 and M2
        meS = bnS_s[:, :, 1].rearrange("p (b s) -> p b s", s=2)
        moS = bnS_s[:, :, 4].rearrange("p (b s) -> p b s", s=2)
        ceS = bnS_s[:, :, 2].rearrange("p (b s) -> p b s", s=2)
        coS = bnS_s[:, :, 5].rearrange("p (b s) -> p b s", s=2)
        sAS = ft("sAS")
        V.tensor_add(sAS[:], ceS, coS)
        uS = ft("uS")
        V.tensor_mul(uS[:], meS, meS)
        vS = ft("vS")
        V.tensor_mul(vS[:], moS, moS)
        wS = ft("wS")
        V.tensor_add(wS[:], uS[:], vS[:])
        sqS = ft("sqS")          # sum y^2
        V.scalar_tensor_tensor(sqS[:], wS[:], float(T // 2), sAS[:],
                               op0=ALU.mult, op1=ALU.add)
        musS = ft("musS")
        V.tensor_add(musS[:], meS, moS)
        qS = ft("qS")
        V.tensor_mul(qS[:], musS[:], musS[:])
        M2S = ft("M2S")          # sum (y - mu)^2
        V.scalar_tensor_tensor(M2S[:], qS[:], -float(T) / 4.0, sqS[:],
                               op0=ALU.mult, op1=ALU.add)

        # var_ratio = log1p(m2F / (M2S + 255e-4))  [Ln later]
        vr = ft("vr")
        V.tensor_scalar(vr[:], M2S[:], (T - 1) * 1e-4, None, op0=ALU.add)
        V.reciprocal(vr[:], vr[:])
        vra = ft("vra")
        V.tensor_mul(vra[:], m2F[:], vr[:])

        # gyro bn-derived per channel -> M2 summed over the 3 channels
        meG = bnG_s[:, :, 1]     # [P, NG*3] strided
        moG = bnG_s[:, :, 4]
        ceG = bnG_s[:, :, 2]
        coG = bnG_s[:, :, 5]
        NG3 = NG * 3
        sAG = ft("sAG", (P, NG3))
        V.tensor_add(sAG[:], ceG, coG)
        uG = ft("uG", (P, NG3))
        V.tensor_mul(uG[:], meG, meG)
        vG = ft("vG", (P, NG3))
        V.tensor_mul(vG[:], moG, moG)
        wG = ft("wG", (P, NG3))
        V.tensor_add(wG[:], uG[:], vG[:])
        sqG = ft("sqG", (P, NG3))
        V.scalar_tensor_tensor(sqG[:], wG[:], float(T // 2), sAG[:],
                               op0=ALU.mult, op1=ALU.add)
        musG = ft("musG", (P, NG3))
        V.tensor_add(musG[:], meG, moG)
        qG = ft("qG", (P, NG3))
        V.tensor_mul(qG[:], musG[:], musG[:])
        M2G = ft("M2G", (P, NG3))
        V.scalar_tensor_tensor(M2G[:], qG[:], -float(T) / 4.0, sqG[:],
                               op0=ALU.mult, op1=ALU.add)
        gM2 = ft("gM2", (P, NG))
        V.reduce_sum(gM2[:], M2G[:].rearrange("p (g c) -> p g c", c=3), axis=AX.X)

        vq = out_t[:, :, 20:32].rearrange("p b (j q) -> p b j q", q=4)

        # ---- Sqrt-set ACT ops ----
        SC.activation(out_t[:, :, 8:10], m2F[:], AF.Sqrt, scale=1.0 / (T - 1))
        rmsF = ft("rmsF")
        SC.activation(rmsF[:], sqF[:], AF.Sqrt, scale=1.0 / T)
        rmsS = ft("rmsS")
        SC.activation(rmsS[:], sqS[:], AF.Sqrt, scale=1.0 / T)

        # kurt/skew (uses std at out[...,8:10])
        sg = ft("sg")
        V.tensor_scalar(sg[:], out_t[:, :, 8:10], 1e-6, None, op0=ALU.max)
        vv = ft("vv")
        V.tensor_mul(vv[:], sg[:], sg[:])
        v4 = ft("v4")
        V.tensor_mul(v4[:], vv[:], vv[:])
        V.reciprocal(v4[:], v4[:])
        kr = ft("kr")
        V.scalar_tensor_tensor(kr[:], v2(m4_s), 1.0 / T, v4[:],
                               op0=ALU.mult, op1=ALU.mult)
        V.tensor_scalar(out_t[:, :, 38:40], kr[:], 30.0, -10.0,
                        op0=ALU.min, op1=ALU.max)
        v3 = ft("v3")
        V.tensor_mul(v3[:], vv[:], sg[:])
        V.reciprocal(v3[:], v3[:])
        sk = ft("sk")
        V.scalar_tensor_tensor(sk[:], v2(m3_s), 1.0 / T, v3[:],
                               op0=ALU.mult, op1=ALU.mult)
        V.tensor_scalar(out_t[:, :, 40:42], sk[:], 10.0, -10.0,
                        op0=ALU.min, op1=ALU.max)

        # trans arg = rmsS / (rmsF + 1e-6)
        rdn = ft("rdn")
        V.tensor_scalar(rdn[:], rmsF[:], EPS, None, op0=ALU.add)
        V.reciprocal(rdn[:], rdn[:])
        targ = ft("targ")
        V.tensor_mul(targ[:], rmsS[:], rdn[:])

        # ---- Ln-set ACT ops (log1p via bias=1) ----
        SC.activation(out_t[:, :, 4:6], ratio_arg[:], AF.Ln, bias=1.0)
        SC.activation(out_t[:, :, 14:16], vra[:], AF.Ln, bias=1.0)
        SC.activation(out_t[:, :, 36:38], targ[:], AF.Ln, bias=1.0)
        SC.activation(vq[:, :, :, 0:2],
                      gM2[:].rearrange("p (b j l) -> p b j l", j=3, l=2),
                      AF.Ln, scale=1.0 / (T - 1), bias=1.0)

        # asym_gy = |fg_var - sg_var| (after log1p)
        ag = ft("ag")
        V.tensor_sub(ag[:], out_t[:, :, 20:22], out_t[:, :, 24:26])
        SC.activation(out_t[:, :, 34:36], ag[:], AF.Abs)

        # ---- store ----
        nc.sync.dma_start(out=out_d.ap().rearrange("(b p) f -> p b f", p=P),
                          in_=out_t[:])


_NC_CACHE = None
_CONSTS = None


def _get_nc():
    global _NC_CACHE, _CONSTS
    if _NC_CACHE is None:
        _NC_CACHE = build_nc()
    if _CONSTS is None:
        _CONSTS = build_consts()
    return _NC_CACHE, _CONSTS


def run(foot, shank, thigh, **kw):
    foot = np.asarray(foot, dtype=np.float32)
    shank = np.asarray(shank, dtype=np.float32)
    thigh = np.asarray(thigh, dtype=np.float32)
    X = np.empty((B_FULL, NCH, T), dtype=np.float32)
    X[:, 0:8] = foot[:, CH_FOOT]
    X[:, 8:16] = shank[:, CH_SHANK]
    X[:, 16:22] = thigh[:, CH_THIGH]

    nc, consts = _get_nc()
    in_maps = []
    for i in range(N_CORES):
        in_maps.append({
            "x": np.ascontiguousarray(X[i * BC:(i + 1) * BC]),
            "w": consts["w"], "wh": consts["wh"], "ident": consts["ident"],
        })
    return run_bass_kernel_spmd(nc, in_maps, core_ids=list(range(N_CORES)),
                                **kw)


def kernel(foot, shank, thigh):
    res = run(foot, shank, thigh)
    return np.concatenate([res.results[i]["out"] for i in range(N_CORES)],
                          axis=0)


# revision 26
# speedup vs baseline: 1.1602x; 1.0040x over previous
"""BioMech feature extraction on Trainium2: 8 NeuronCores, pure data-parallel SPMD.

Self-contained: takes full inputs foot/shank/thigh [8192, 12, 256] fp32,
returns [8192, 44] fp32 feature matrix matching the reference stack order.

Strategy per core (1024 samples, 8 blocks of 128 partitions):
  - Host packs the 22 used channels -> X [B, 22, 256] (foot z+gyro, shank
    z+gyro, thigh gyro), one DMA per 128-sample block.
  - FFT features via a single merged PE matmul with DFT weights
    W = [C(k=0..128) | S(k=1..128) | C*sqrt(k) | S*sqrt(k)] (512 cols,
    exactly one PSUM bank), after a PE transpose of the foot-z channels.
    Sum x^2 comes from Parseval instead of a separate pass.
  - Sums/variances via DVE bn_stats (channel-paired); peaks via fused
    multi-channel abs-max reductions; counts via tensor_scalar
    cache-reduce; ACT Square/Abs/Copy with sum-accumulators for moments.
  - Elementwise products (zcr product, vib diff, c^3, centering) on GpSimd.
  - Per-sample scalar math batched over all blocks at the end.
"""

import contextlib

import numpy as np

import concourse.bacc as bacc
import concourse.tile as tile
import concourse.mybir as mybir
from concourse.bass_utils import run_bass_kernel_spmd

F32 = mybir.dt.float32
AF = mybir.ActivationFunctionType
ALU = mybir.AluOpType
AX = mybir.AxisListType

N_CORES = 8
B_FULL = 8192
T = 256
P = 128
BC = B_FULL // N_CORES          # 1024 samples per core
NBLK = BC // P                  # 8 blocks
NCH = 22
NBINS = 129
HF_BIN = 60
NW = 512                        # merged DFT weight columns
EPS = 1e-6

CH_FOOT = [2, 3, 4, 5, 8, 9, 10, 11]
CH_SHANK = [2, 3, 4, 5, 8, 9, 10, 11]
CH_THIGH = [3, 4, 5, 9, 10, 11]

FZ = (0, 4)          # packed idx of foot z lt/rt
SZ = (8, 12)         # packed idx of shank z lt/rt


def build_consts():
    t = np.arange(T, dtype=np.float64)
    k = np.arange(NBINS, dtype=np.float64)
    ang = 2.0 * np.pi * np.outer(t, k) / T
    C = np.cos(ang)                      # [256, 129] k=0..128
    S = np.sin(ang)
    sk = np.sqrt(k)
    # merged: [C(0..128) | S(1..128) | C*sqrt(k)(1..128) | S*sqrt(k)(1..127)]
    w = np.concatenate([C, S[:, 1:129], (C * sk)[:, 1:129],
                        (S * sk)[:, 1:128]], axis=1)       # [256, 512]
    assert w.shape[1] == NW
    w = np.ascontiguousarray(w.reshape(2, P, NW), dtype=np.float32)
    wh = np.zeros((2, P, 2), dtype=np.float32)
    wh[0, :, 0] = 1.0   # chunk 0 (t<128) -> first-half abs sum
    wh[1, :, 1] = 1.0   # chunk 1 (t>=128) -> second-half abs sum
    ident = np.eye(P, dtype=np.float32)
    return {"w": w, "wh": wh, "ident": ident}


def build_nc():
    nc = bacc.Bacc("TRN2", target_bir_lowering=False, debug=False,
                   num_devices=N_CORES)
    x_d = nc.dram_tensor("x", [BC, NCH, T], F32, kind="ExternalInput")
    w_d = nc.dram_tensor("w", [2, P, NW], mybir.dt.float32r,
                         kind="ExternalInput")
    wh_d = nc.dram_tensor("wh", [2, P, 2], F32, kind="ExternalInput")
    id_d = nc.dram_tensor("ident", [P, P], F32, kind="ExternalInput")
    out_d = nc.dram_tensor("out", [BC, 44], F32, kind="ExternalOutput")

    with tile.TileContext(nc) as tc:
        _body(tc, x_d, w_d, wh_d, id_d, out_d)
    nc.compile()
    return nc


def _body(tc, x_d, w_d, wh_d, id_d, out_d):
    nc = tc.nc
    NS = NBLK * 2        # (block, side) stat slots
    NG = NBLK * 6        # (block, group) slots

    ctx = contextlib.ExitStack()
    with ctx:
        pers = ctx.enter_context(tc.tile_pool(name="pers", bufs=1))
        p_in = ctx.enter_context(tc.tile_pool(name="xin", bufs=4))
        p_psxt = ctx.enter_context(tc.tile_pool(name="psxt", bufs=2, space="PSUM"))
        p_psmm = ctx.enter_context(tc.tile_pool(name="psmm", bufs=3, space="PSUM"))
        p_psh = ctx.enter_context(tc.tile_pool(name="psh", bufs=2, space="PSUM"))
        p_xt = ctx.enter_context(tc.tile_pool(name="xt", bufs=3))
        p_p2 = ctx.enter_context(tc.tile_pool(name="p2", bufs=3))
        p_scr = ctx.enter_context(tc.tile_pool(name="scr", bufs=4))
        p_junk = ctx.enter_context(tc.tile_pool(name="junk", bufs=8))
        p_small = ctx.enter_context(tc.tile_pool(name="small", bufs=3))
        fin = ctx.enter_context(tc.tile_pool(name="fin", bufs=1))

        # ---- constants to SBUF ----
        w_sb = pers.tile([P, 2, NW], mybir.dt.float32r, tag="w", name="w_sb")
        wh_sb = pers.tile([P, 2, 2], F32, tag="wh", name="wh_sb")
        id_sb = pers.tile([P, P], F32, tag="ident", name="id_sb")
        nc.sync.dma_start(out=w_sb[:], in_=w_d.ap().rearrange("j p n -> p j n"))
        nc.sync.dma_start(out=wh_sb[:], in_=wh_d.ap().rearrange("j p n -> p j n"))
        nc.sync.dma_start(out=id_sb[:], in_=id_d.ap())

        def stat(tag, n=NS):
            return pers.tile([P, n], F32, tag=tag, name=tag)

        tot_s = stat("tot")     # sum power (129 bins)
        scn_s = stat("scn")     # sum k*power
        hf_s = stat("hf")       # sum power k>=60 (cos part)
        muN_s = stat("muN", 2 * NS)  # [mean fz, alt-sum/256] per (b, side)
        m3_s = stat("m3")       # sum (fz-mu)^3
        m4_s = stat("m4")       # sum (fz-mu)^4
        imp_s = stat("imp")     # count |fz| >= 0.3 pk
        zc_s = stat("zc")       # zero crossing count
        vib_s = stat("vib")     # sum |diff sz|
        habs_s = stat("habs", 2 * NS)          # [b, side, half]
        bnS_s = pers.tile([P, NS, 6], F32, tag="bnS", name="bnS")
        bnG_s = pers.tile([P, NG * 3, 6], F32, tag="bnG", name="bnG")
        out_t = pers.tile([P, NBLK, 44], F32, tag="out", name="out_t")

        x_ap = x_d.ap()

        for b in range(NBLK):
            Xb = p_in.tile([P, NCH, T], F32, tag="xb", name="Xb")
            nc.sync.dma_start(out=Xb[:, 0:8, :],
                              in_=x_ap[b * P:(b + 1) * P, 0:8, :])
            nc.sync.dma_start(out=Xb[:, 8:NCH, :],
                              in_=x_ap[b * P:(b + 1) * P, 8:NCH, :])

            # fz peaks for both sides in one fused reduce [p, 2, 256]
            # (emitted before the channel loop: thr below reads pkF_s)
            fzv = Xb[:, 0:8, :].rearrange("p (s c) t -> p s c t", s=2)[:, :, 0, :]
            nc.vector.tensor_reduce(out_t[:, b, 0:2], fzv, axis=AX.X,
                                    op=ALU.max, apply_absolute_value=True)

            # ---------- foot z channels ----------
            for side, ch in enumerate(FZ):
                x = Xb[:, ch, :]
                i2 = 2 * b + side
                xt_ps = p_psxt.tile([P, 2, P], F32, tag="xtps", name="xt_ps")
                nc.tensor.transpose(xt_ps[:, 0, :], x[:, 0:P], id_sb[:])
                nc.tensor.transpose(xt_ps[:, 1, :], x[:, P:T], id_sb[:])
                XT = p_xt.tile([P, 2, P], mybir.dt.float32r, tag="xt",
                               name="XT")
                nc.scalar.copy(XT[:], xt_ps[:])
                AXT = p_xt.tile([P, 2, P], F32, tag="axt", name="AXT")
                nc.scalar.activation(AXT[:], xt_ps[:], AF.Abs)

                psF = p_psmm.tile([P, NW], F32, tag="psF", name="psF")
                psH = p_psh.tile([P, 2], F32, tag="psH", name="psH")
                for j in (0, 1):
                    nc.tensor.matmul(psF[:], XT[:, j, :], w_sb[:, j, :],
                                     start=(j == 0), stop=(j == 1))
                for j in (0, 1):
                    nc.tensor.matmul(psH[:], AXT[:, j, :], wh_sb[:, j, :],
                                     start=(j == 0), stop=(j == 1))

                P2 = p_p2.tile([P, 257], F32, tag="p2", name="P2")
                nc.scalar.activation(P2[:], psF[:, 0:257], AF.Square,
                                     accum_out=tot_s[:, i2:i2 + 1])
                jB = p_junk.tile([P, 255], F32, tag="junkb", name="jB")
                nc.scalar.activation(jB[:], psF[:, 257:NW], AF.Square,
                                     accum_out=scn_s[:, i2:i2 + 1])
                # hf sum over {cos k>=60} u {sin k>=60}: two 69-wide ranges
                # 128 apart -> one strided reduce
                import concourse.bass as _bass
                p2b = P2[:]
                hfap = _bass.AP(tensor=p2b.tensor,
                                offset=p2b.offset + HF_BIN,
                                ap=[p2b.ap[0], [P, 2], [1, NBINS - HF_BIN]])
                nc.vector.reduce_sum(hf_s[:, i2:i2 + 1], hfap, axis=AX.XY)
                nc.scalar.copy(habs_s[:, 2 * i2:2 * i2 + 2], psH[:])
                # cols 0 and 128 of psF = sum(x) and alternating sum -> both
                # scaled by 1/256: mu and aN (Parseval: p0 = (256 mu)^2 etc.)
                nc.scalar.activation(muN_s[:, 2 * i2:2 * i2 + 2],
                                     psF[:, 0:NBINS:P], AF.Copy,
                                     scale=-1.0 / T)

                ABSX = p_scr.tile([P, T], F32, tag="absx", name="ABSX")
                nc.scalar.activation(ABSX[:], x, AF.Abs)
                thr = p_small.tile([P, 1], F32, tag="thr", name="thr")
                nc.vector.tensor_scalar(thr[:], out_t[:, b, side:side + 1],
                                        -0.3, None, op0=ALU.mult)
                j1 = p_junk.tile([P, T], F32, tag="junk", name="j1")
                # sum sign(|x| - 0.3pk); count_above = (256 + S)/2
                nc.scalar.activation(j1[:], ABSX[:], AF.Sign, bias=thr[:],
                                     accum_out=imp_s[:, i2:i2 + 1])

                C = p_scr.tile([P, T], F32, tag="c", name="C")
                nc.scalar.activation(C[:], x, AF.Identity,
                                     bias=muN_s[:, 2 * i2:2 * i2 + 1])
                C2 = p_scr.tile([P, T], F32, tag="c2", name="C2")
                nc.scalar.activation(C2[:], C[:], AF.Square)
                j3 = p_junk.tile([P, T], F32, tag="junk", name="j3")
                nc.scalar.activation(j3[:], C2[:], AF.Square,
                                     accum_out=m4_s[:, i2:i2 + 1])
                C3 = p_scr.tile([P, T], F32, tag="c3", name="C3")
                nc.gpsimd.tensor_tensor(C3[:], C2[:], C[:], op=ALU.mult)
                j2 = p_junk.tile([P, T], F32, tag="junk", name="j2")
                nc.scalar.activation(j2[:], C3[:], AF.Copy,
                                     accum_out=m3_s[:, i2:i2 + 1])

                PR = p_scr.tile([P, T - 1], F32, tag="pr", name="PR")
                nc.gpsimd.tensor_tensor(PR[:], x[:, 1:T], x[:, 0:T - 1],
                                        op=ALU.mult)
                j4 = p_junk.tile([P, T], F32, tag="junk", name="j4")
                # sum sign(x_t x_{t+1}); crossings = (255 - S)/2
                nc.scalar.activation(j4[:, 0:T - 1], PR[:], AF.Sign,
                                     accum_out=zc_s[:, i2:i2 + 1])

            # ---------- shank z channels ----------
            szv = Xb[:, 8:16, :].rearrange("p (s c) t -> p s c t", s=2)[:, :, 0, :]
            nc.vector.tensor_reduce(out_t[:, b, 2:4], szv, axis=AX.X,
                                    op=ALU.max, apply_absolute_value=True)
            for side, ch in enumerate(SZ):
                nc.vector.bn_stats(bnS_s[:, 2 * b + side, :], Xb[:, ch, :])
                y = Xb[:, ch, :]
                i2 = 2 * b + side
                D = p_scr.tile([P, T - 1], F32, tag="d", name="D")
                nc.gpsimd.tensor_tensor(D[:], y[:, 1:T], y[:, 0:T - 1],
                                        op=ALU.subtract)
                j6 = p_junk.tile([P, T], F32, tag="junk", name="j6")
                nc.scalar.activation(j6[:, 0:T - 1], D[:], AF.Abs,
                                     accum_out=vib_s[:, i2:i2 + 1])

            # ---------- gyro groups ----------
            # peaks: foot+shank groups (start ch 1,5,9,13) in one reduce,
            # thigh groups (16..21) in another
            gfs = Xb[:, 1:17, :].rearrange("p (g c) t -> p g c t", g=4)[:, :, 0:3, :]
            vqb = out_t[:, b, 20:32].rearrange("p (j q) -> p j q", q=4)
            nc.vector.tensor_reduce(vqb[:, 0:2, 2:4], gfs, axis=AX.XY,
                                    op=ALU.max, apply_absolute_value=True)
            gth = Xb[:, 16:22, :].rearrange("p (g c) t -> p g c t", g=2)
            nc.vector.tensor_reduce(vqb[:, 2, 2:4], gth, axis=AX.XY,
                                    op=ALU.max, apply_absolute_value=True)
            for gi, c0 in enumerate((1, 5, 9, 13, 16, 19)):
                ig = 6 * b + gi
                for cc in range(3):
                    nc.vector.bn_stats(bnG_s[:, ig * 3 + cc, :],
                                       Xb[:, c0 + cc, :])

        # ================= final batched scalar phase =================
        def v2(tbl):
            return tbl[:].rearrange("p (b s) -> p b s", s=2)

        def ft(tag, shape=(P, NBLK, 2)):
            return fin.tile(list(shape), F32, tag=tag, name=tag)

        V = nc.vector
        SC = nc.scalar

        # ratio = log1p(f_pk / (s_pk + 1e-4))  [Ln later]
        r_spk = ft("r_spk")
        V.tensor_scalar(r_spk[:], out_t[:, :, 2:4], 1e-4, None, op0=ALU.add)
        V.reciprocal(r_spk[:], r_spk[:])
        ratio_arg = ft("ratio_arg")
        V.tensor_mul(ratio_arg[:], out_t[:, :, 0:2], r_spk[:])

        # total power reciprocal (shared by hf and sc)
        r_tot = ft("r_tot")
        V.tensor_scalar(r_tot[:], v2(tot_s), EPS, None, op0=ALU.add)
        V.reciprocal(r_tot[:], r_tot[:])
        V.tensor_mul(out_t[:, :, 6:8], v2(hf_s), r_tot[:])
        V.scalar_tensor_tensor(out_t[:, :, 16:18], v2(scn_s), 1.0 / NBINS,
                               r_tot[:], op0=ALU.mult, op1=ALU.mult)

        # dur, vib, zcr scaled counts
        V.tensor_scalar(out_t[:, :, 18:20], v2(imp_s), 1.0 / (2 * T), 0.5,
                        op0=ALU.mult, op1=ALU.add)
        V.tensor_scalar(out_t[:, :, 12:14], v2(vib_s), 1.0 / (T - 1), None, op0=ALU.mult)
        V.tensor_scalar(out_t[:, :, 42:44], v2(zc_s), -1.0 / (2 * (T - 1)),
                        0.5, op0=ALU.mult, op1=ALU.add)

        # decay = h0 / (h1 + 128e-6)
        hv = habs_s[:].rearrange("p (b s h) -> p b s h", s=2, h=2)
        dden = ft("dden")
        V.tensor_scalar(dden[:], hv[:, :, :, 1], (T // 2) * EPS, None, op0=ALU.add)
        V.reciprocal(dden[:], dden[:])
        V.tensor_mul(out_t[:, :, 10:12], hv[:, :, :, 0], dden[:])

        # asym_acc = |f_pk - s_pk|
        aa = ft("aa")
        V.tensor_sub(aa[:], out_t[:, :, 0:2], out_t[:, :, 2:4])
        SC.activation(out_t[:, :, 32:34], aa[:], AF.Abs)

        # foot m2 via Parseval:
        # sum x^2 = (2 tot - p0 - pN)/256 with p0 = (256 mu)^2, pN = (256 aN)^2
        #         = tot/128 - 256 (mu^2 + aN^2)
        mv = muN_s[:].rearrange("p (b s h) -> p b s h", s=2, h=2)
        qF = ft("qF")
        V.tensor_mul(qF[:], mv[:, :, :, 0], mv[:, :, :, 0])
        qN = ft("qN")
        V.tensor_mul(qN[:], mv[:, :, :, 1], mv[:, :, :, 1])
        qsum = ft("qsum")
        V.tensor_add(qsum[:], qF[:], qN[:])
        tq = ft("tq")
        V.tensor_scalar(tq[:], v2(tot_s), 1.0 / (T // 2), None, op0=ALU.mult)
        sqF = ft("sqF")           # sum x^2
        V.scalar_tensor_tensor(sqF[:], qsum[:], -float(T), tq[:],
                               op0=ALU.mult, op1=ALU.add)
        m2F = ft("m2F")           # sum (x-mu)^2
        V.scalar_tensor_tensor(m2F[:], qF[:], -float(T), sqF[:],
                               op0=ALU.mult, op1=ALU.add)

        # shank bn-derived: sum y^2 and M2
        meS = bnS_s[:, :, 1].rearrange("p (b s) -> p b s", s=2)
        moS = bnS_s[:, :, 4].rearrange("p (b s) -> p b s", s=2)
        ceS = bnS_s[:, :, 2].rearrange("p (b s) -> p b s", s=2)
        coS = bnS_s[:, :, 5].rearrange("p (b s) -> p b s", s=2)
        sAS = ft("sAS")
        V.tensor_add(sAS[:], ceS, coS)
        uS = ft("uS")
        V.tensor_mul(uS[:], meS, meS)
        vS = ft("vS")
        V.tensor_mul(vS[:], moS, moS)
        wS = ft("wS")
        V.tensor_add(wS[:], uS[:], vS[:])
        sqS = ft("sqS")          # sum y^2
        V.scalar_tensor_tensor(sqS[:], wS[:], float(T // 2), sAS[:],
                               op0=ALU.mult, op1=ALU.add)
        musS = ft("musS")
        V.tensor_add(musS[:], meS, moS)
        qS = ft("qS")
        V.tensor_mul(qS[:], musS[:], musS[:])
        M2S = ft("M2S")          # sum (y - mu)^2
        V.scalar_tensor_tensor(M2S[:], qS[:], -float(T) / 4.0, sqS[:],
                               op0=ALU.mult, op1=ALU.add)

        # var_ratio = log1p(m2F / (M2S + 255e-4))  [Ln later]
        vr = ft("vr")
        V.tensor_scalar(vr[:], M2S[:], (T - 1) * 1e-4, None, op0=ALU.add)
        V.reciprocal(vr[:], vr[:])
        vra = ft("vra")
        V.tensor_mul(vra[:], m2F[:], vr[:])

        # gyro bn-derived per channel -> M2 summed over the 3 channels
        meG = bnG_s[:, :, 1]     # [P, NG*3] strided
        moG = bnG_s[:, :, 4]
        ceG = bnG_s[:, :, 2]
        coG = bnG_s[:, :, 5]
        NG3 = NG * 3
        sAG = ft("sAG", (P, NG3))
        V.tensor_add(sAG[:], ceG, coG)
        uG = ft("uG", (P, NG3))
        V.tensor_mul(uG[:], meG, meG)
        vG = ft("vG", (P, NG3))
        V.tensor_mul(vG[:], moG, moG)
        wG = ft("wG", (P, NG3))
        V.tensor_add(wG[:], uG[:], vG[:])
        sqG = ft("sqG", (P, NG3))
        V.scalar_tensor_tensor(sqG[:], wG[:], float(T // 2), sAG[:],
                               op0=ALU.mult, op1=ALU.add)
        musG = ft("musG", (P, NG3))
        V.tensor_add(musG[:], meG, moG)
        qG = ft("qG", (P, NG3))
        V.tensor_mul(qG[:], musG[:], musG[:])
        M2G = ft("M2G", (P, NG3))
        V.scalar_tensor_tensor(M2G[:], qG[:], -float(T) / 4.0, sqG[:],
                               op0=ALU.mult, op1=ALU.add)
        gM2 = ft("gM2", (P, NG))
        V.reduce_sum(gM2[:], M2G[:].rearrange("p (g c) -> p g c", c=3), axis=AX.X)

        vq = out_t[:, :, 20:32].rearrange("p b (j q) -> p b j q", q=4)

        # ---- Sqrt-set ACT ops ----
        SC.activation(out_t[:, :, 8:10], m2F[:], AF.Sqrt, scale=1.0 / (T - 1))
        rmsF = ft("rmsF")
        SC.activation(rmsF[:], sqF[:], AF.Sqrt, scale=1.0 / T)
        rmsS = ft("rmsS")
        SC.activation(rmsS[:], sqS[:], AF.Sqrt, scale=1.0 / T)

        # kurt/skew (uses std at out[...,8:10])
        sg = ft("sg")
        V.tensor_scalar(sg[:], out_t[:, :, 8:10], 1e-6, None, op0=ALU.max)
        vv = ft("vv")
        V.tensor_mul(vv[:], sg[:], sg[:])
        v4 = ft("v4")
        V.tensor_mul(v4[:], vv[:], vv[:])
        V.reciprocal(v4[:], v4[:])
        kr = ft("kr")
        V.scalar_tensor_tensor(kr[:], v2(m4_s), 1.0 / T, v4[:],
                               op0=ALU.mult, op1=ALU.mult)
        V.tensor_scalar(out_t[:, :, 38:40], kr[:], 30.0, -10.0,
                        op0=ALU.min, op1=ALU.max)
        v3 = ft("v3")
        V.tensor_mul(v3[:], vv[:], sg[:])
        V.reciprocal(v3[:], v3[:])
        sk = ft("sk")
        V.scalar_tensor_tensor(sk[:], v2(m3_s), 1.0 / T, v3[:],
                               op0=ALU.mult, op1=ALU.mult)
        V.tensor_scalar(out_t[:, :, 40:42], sk[:], 10.0, -10.0,
                        op0=ALU.min, op1=ALU.max)

        # trans arg = rmsS / (rmsF + 1e-6)
        rdn = ft("rdn")
        V.tensor_scalar(rdn[:], rmsF[:], EPS, None, op0=ALU.add)
        V.reciprocal(rdn[:], rdn[:])
        targ = ft("targ")
        V.tensor_mul(targ[:], rmsS[:], rdn[:])

        # ---- Ln-set ACT ops (log1p via bias=1) ----
        SC.activation(out_t[:, :, 4:6], ratio_arg[:], AF.Ln, bias=1.0)
        SC.activation(out_t[:, :, 14:16], vra[:], AF.Ln, bias=1.0)
        SC.activation(out_t[:, :, 36:38], targ[:], AF.Ln, bias=1.0)
        SC.activation(vq[:, :, :, 0:2],
                      gM2[:].rearrange("p (b j l) -> p b j l", j=3, l=2),
                      AF.Ln, scale=1.0 / (T - 1), bias=1.0)

        # asym_gy = |fg_var - sg_var| (after log1p)
        ag = ft("ag")
        V.tensor_sub(ag[:], out_t[:, :, 20:22], out_t[:, :, 24:26])
        SC.activation(out_t[:, :, 34:36], ag[:], AF.Abs)

        # ---- store ----
        nc.sync.dma_start(out=out_d.ap().rearrange("(b p) f -> p b f", p=P),
                          in_=out_t[:])


_NC_CACHE = None
_CONSTS = None


def _get_nc():
    global _NC_CACHE, _CONSTS
    if _NC_CACHE is None:
        _NC_CACHE = build_nc()
    if _CONSTS is None:
        _CONSTS = build_consts()
    return _NC_CACHE, _CONSTS


def run(foot, shank, thigh, **kw):
    foot = np.asarray(foot, dtype=np.float32)
    shank = np.asarray(shank, dtype=np.float32)
    thigh = np.asarray(thigh, dtype=np.float32)
    X = np.empty((B_FULL, NCH, T), dtype=np.float32)
    X[:, 0:8] = foot[:, CH_FOOT]
    X[:, 8:16] = shank[:, CH_SHANK]
    X[:, 16:22] = thigh[:, CH_THIGH]

    nc, consts = _get_nc()
    in_maps = []
    for i in range(N_CORES):
        in_maps.append({
            "x": np.ascontiguousarray(X[i * BC:(i + 1) * BC]),
            "w": consts["w"], "wh": consts["wh"], "ident": consts["ident"],
        })
    return run_bass_kernel_spmd(nc, in_maps, core_ids=list(range(N_CORES)),
                                **kw)


def kernel(foot, shank, thigh):
    res = run(foot, shank, thigh)
    return np.concatenate([res.results[i]["out"] for i in range(N_CORES)],
                          axis=0)


# revision 27
# speedup vs baseline: 1.1691x; 1.0077x over previous
"""BioMech feature extraction on Trainium2: 8 NeuronCores, pure data-parallel SPMD.

Self-contained: takes full inputs foot/shank/thigh [8192, 12, 256] fp32,
returns [8192, 44] fp32 feature matrix matching the reference stack order.

Strategy per core (1024 samples, 8 blocks of 128 partitions):
  - Host packs the 22 used channels -> X [B, 22, 256] (foot z+gyro, shank
    z+gyro, thigh gyro), one DMA per 128-sample block.
  - FFT features via a single merged PE matmul with DFT weights
    W = [C(k=0..128) | S(k=1..128) | C*sqrt(k) | S*sqrt(k)] (512 cols,
    exactly one PSUM bank), after a PE transpose of the foot-z channels.
    Sum x^2 comes from Parseval instead of a separate pass.
  - Sums/variances via DVE bn_stats (channel-paired); peaks via fused
    multi-channel abs-max reductions; counts via tensor_scalar
    cache-reduce; ACT Square/Abs/Copy with sum-accumulators for moments.
  - Elementwise products (zcr product, vib diff, c^3, centering) on GpSimd.
  - Per-sample scalar math batched over all blocks at the end.
"""

import contextlib

import numpy as np

import concourse.bacc as bacc
import concourse.tile as tile
import concourse.mybir as mybir
from concourse.bass_utils import run_bass_kernel_spmd

F32 = mybir.dt.float32
AF = mybir.ActivationFunctionType
ALU = mybir.AluOpType
AX = mybir.AxisListType

N_CORES = 8
B_FULL = 8192
T = 256
P = 128
BC = B_FULL // N_CORES          # 1024 samples per core
NBLK = BC // P                  # 8 blocks
NCH = 22
NBINS = 129
HF_BIN = 60
NW = 512                        # merged DFT weight columns
EPS = 1e-6

CH_FOOT = [2, 3, 4, 5, 8, 9, 10, 11]
CH_SHANK = [2, 3, 4, 5, 8, 9, 10, 11]
CH_THIGH = [3, 4, 5, 9, 10, 11]

FZ = (0, 4)          # packed idx of foot z lt/rt
SZ = (8, 12)         # packed idx of shank z lt/rt


def build_consts():
    t = np.arange(T, dtype=np.float64)
    k = np.arange(NBINS, dtype=np.float64)
    ang = 2.0 * np.pi * np.outer(t, k) / T
    C = np.cos(ang)                      # [256, 129] k=0..128
    S = np.sin(ang)
    sk = np.sqrt(k)
    # merged: [C(0..128) | S(1..128) | C*sqrt(k)(1..128) | S*sqrt(k)(1..127)]
    w = np.concatenate([C, S[:, 1:129], (C * sk)[:, 1:129],
                        (S * sk)[:, 1:128]], axis=1)       # [256, 512]
    assert w.shape[1] == NW
    w = np.ascontiguousarray(w.reshape(2, P, NW), dtype=np.float32)
    wh = np.zeros((2, P, 2), dtype=np.float32)
    wh[0, :, 0] = 1.0   # chunk 0 (t<128) -> first-half abs sum
    wh[1, :, 1] = 1.0   # chunk 1 (t>=128) -> second-half abs sum
    ident = np.eye(P, dtype=np.float32)
    return {"w": w, "wh": wh, "ident": ident}


def build_nc():
    nc = bacc.Bacc("TRN2", target_bir_lowering=False, debug=False,
                   num_devices=N_CORES)
    x_d = nc.dram_tensor("x", [BC, NCH, T], F32, kind="ExternalInput")
    w_d = nc.dram_tensor("w", [2, P, NW], mybir.dt.float32r,
                         kind="ExternalInput")
    wh_d = nc.dram_tensor("wh", [2, P, 2], F32, kind="ExternalInput")
    id_d = nc.dram_tensor("ident", [P, P], F32, kind="ExternalInput")
    out_d = nc.dram_tensor("out", [BC, 44], F32, kind="ExternalOutput")

    with tile.TileContext(nc) as tc:
        _body(tc, x_d, w_d, wh_d, id_d, out_d)
    nc.compile()
    return nc


def _body(tc, x_d, w_d, wh_d, id_d, out_d):
    nc = tc.nc
    NS = NBLK * 2        # (block, side) stat slots
    NG = NBLK * 6        # (block, group) slots

    ctx = contextlib.ExitStack()
    with ctx:
        pers = ctx.enter_context(tc.tile_pool(name="pers", bufs=1))
        p_in = ctx.enter_context(tc.tile_pool(name="xin", bufs=4))
        p_psxt = ctx.enter_context(tc.tile_pool(name="psxt", bufs=2, space="PSUM"))
        p_psmm = ctx.enter_context(tc.tile_pool(name="psmm", bufs=3, space="PSUM"))
        p_psh = ctx.enter_context(tc.tile_pool(name="psh", bufs=2, space="PSUM"))
        p_xt = ctx.enter_context(tc.tile_pool(name="xt", bufs=3))
        p_p2 = ctx.enter_context(tc.tile_pool(name="p2", bufs=3))
        p_scr = ctx.enter_context(tc.tile_pool(name="scr", bufs=4))
        p_junk = ctx.enter_context(tc.tile_pool(name="junk", bufs=8))
        p_small = ctx.enter_context(tc.tile_pool(name="small", bufs=3))
        fin = ctx.enter_context(tc.tile_pool(name="fin", bufs=1))

        # ---- constants to SBUF ----
        w_sb = pers.tile([P, 2, NW], mybir.dt.float32r, tag="w", name="w_sb")
        wh_sb = pers.tile([P, 2, 2], F32, tag="wh", name="wh_sb")
        id_sb = pers.tile([P, P], F32, tag="ident", name="id_sb")
        nc.sync.dma_start(out=w_sb[:], in_=w_d.ap().rearrange("j p n -> p j n"))
        nc.sync.dma_start(out=wh_sb[:], in_=wh_d.ap().rearrange("j p n -> p j n"))
        nc.sync.dma_start(out=id_sb[:], in_=id_d.ap())

        def stat(tag, n=NS):
            return pers.tile([P, n], F32, tag=tag, name=tag)

        tot_s = stat("tot")     # sum power (129 bins)
        scn_s = stat("scn")     # sum k*power
        hf_s = stat("hf")       # sum power k>=60 (cos part)
        muN_s = stat("muN", 2 * NS)  # [mean fz, alt-sum/256] per (b, side)
        m3_s = stat("m3")       # sum (fz-mu)^3
        m4_s = stat("m4")       # sum (fz-mu)^4
        imp_s = stat("imp")     # count |fz| >= 0.3 pk
        zc_s = stat("zc")       # zero crossing count
        vib_s = stat("vib")     # sum |diff sz|
        habs_s = stat("habs", 2 * NS)          # [b, side, half]
        bnS_s = pers.tile([P, NS, 6], F32, tag="bnS", name="bnS")
        bnG_s = pers.tile([P, NG * 3, 6], F32, tag="bnG", name="bnG")
        out_t = pers.tile([P, NBLK, 44], F32, tag="out", name="out_t")

        x_ap = x_d.ap()

        for b in range(NBLK):
            Xb = p_in.tile([P, NCH, T], F32, tag="xb", name="Xb")
            nc.sync.dma_start(out=Xb[:, 0:8, :],
                              in_=x_ap[b * P:(b + 1) * P, 0:8, :])
            nc.sync.dma_start(out=Xb[:, 8:NCH, :],
                              in_=x_ap[b * P:(b + 1) * P, 8:NCH, :])

            # fz peaks for both sides in one fused reduce [p, 2, 256]
            # (emitted before the channel loop: thr below reads pkF_s)
            fzv = Xb[:, 0:8, :].rearrange("p (s c) t -> p s c t", s=2)[:, :, 0, :]
            nc.vector.tensor_reduce(out_t[:, b, 0:2], fzv, axis=AX.X,
                                    op=ALU.max, apply_absolute_value=True)

            # ---------- foot z channels ----------
            # transposes for BOTH sides into one PSUM bank, then single
            # 512-element ACT copy/abs ops (amortize the 224-cycle bubble)
            xt_ps = p_psxt.tile([P, 2, 2, P], F32, tag="xtps", name="xt_ps")
            for side, ch in enumerate(FZ):
                x = Xb[:, ch, :]
                nc.tensor.transpose(xt_ps[:, side, 0, :], x[:, 0:P], id_sb[:])
                nc.tensor.transpose(xt_ps[:, side, 1, :], x[:, P:T], id_sb[:])
            XT = p_xt.tile([P, 2, 2, P], mybir.dt.float32r, tag="xt",
                           name="XT")
            nc.scalar.copy(XT[:], xt_ps[:])
            AXT = p_xt.tile([P, 2, 2, P], F32, tag="axt", name="AXT")
            nc.scalar.activation(AXT[:], xt_ps[:], AF.Abs)
            ABSX2 = p_scr.tile([P, 2, T], F32, tag="absx", name="ABSX2")
            nc.scalar.activation(ABSX2[:], fzv, AF.Abs)

            for side, ch in enumerate(FZ):
                x = Xb[:, ch, :]
                i2 = 2 * b + side
                psF = p_psmm.tile([P, NW], F32, tag="psF", name="psF")
                psH = p_psh.tile([P, 2], F32, tag="psH", name="psH")
                for j in (0, 1):
                    nc.tensor.matmul(psF[:], XT[:, side, j, :], w_sb[:, j, :],
                                     start=(j == 0), stop=(j == 1))
                for j in (0, 1):
                    nc.tensor.matmul(psH[:], AXT[:, side, j, :], wh_sb[:, j, :],
                                     start=(j == 0), stop=(j == 1))

                P2 = p_p2.tile([P, 257], F32, tag="p2", name="P2")
                nc.scalar.activation(P2[:], psF[:, 0:257], AF.Square,
                                     accum_out=tot_s[:, i2:i2 + 1])
                jB = p_junk.tile([P, 255], F32, tag="junkb", name="jB")
                nc.scalar.activation(jB[:], psF[:, 257:NW], AF.Square,
                                     accum_out=scn_s[:, i2:i2 + 1])
                # hf sum over {cos k>=60} u {sin k>=60}: two 69-wide ranges
                # 128 apart -> one strided reduce
                import concourse.bass as _bass
                p2b = P2[:]
                hfap = _bass.AP(tensor=p2b.tensor,
                                offset=p2b.offset + HF_BIN,
                                ap=[p2b.ap[0], [P, 2], [1, NBINS - HF_BIN]])
                nc.vector.reduce_sum(hf_s[:, i2:i2 + 1], hfap, axis=AX.XY)
                nc.scalar.copy(habs_s[:, 2 * i2:2 * i2 + 2], psH[:])
                # cols 0 and 128 of psF = sum(x) and alternating sum -> both
                # scaled by 1/256: mu and aN (Parseval: p0 = (256 mu)^2 etc.)
                nc.scalar.activation(muN_s[:, 2 * i2:2 * i2 + 2],
                                     psF[:, 0:NBINS:P], AF.Copy,
                                     scale=-1.0 / T)

                thr = p_small.tile([P, 1], F32, tag="thr", name="thr")
                nc.vector.tensor_scalar(thr[:], out_t[:, b, side:side + 1],
                                        -0.3, None, op0=ALU.mult)
                j1 = p_junk.tile([P, T], F32, tag="junk", name="j1")
                # sum sign(|x| - 0.3pk); count_above = (256 + S)/2
                nc.scalar.activation(j1[:], ABSX2[:, side, :], AF.Sign,
                                     bias=thr[:],
                                     accum_out=imp_s[:, i2:i2 + 1])

                C = p_scr.tile([P, T], F32, tag="c", name="C")
                nc.scalar.activation(C[:], x, AF.Identity,
                                     bias=muN_s[:, 2 * i2:2 * i2 + 1])
                C2 = p_scr.tile([P, T], F32, tag="c2", name="C2")
                nc.scalar.activation(C2[:], C[:], AF.Square)
                j3 = p_junk.tile([P, T], F32, tag="junk", name="j3")
                nc.scalar.activation(j3[:], C2[:], AF.Square,
                                     accum_out=m4_s[:, i2:i2 + 1])
                C3 = p_scr.tile([P, T], F32, tag="c3", name="C3")
                nc.gpsimd.tensor_tensor(C3[:], C2[:], C[:], op=ALU.mult)
                j2 = p_junk.tile([P, T], F32, tag="junk", name="j2")
                nc.scalar.activation(j2[:], C3[:], AF.Copy,
                                     accum_out=m3_s[:, i2:i2 + 1])

                PR = p_scr.tile([P, T - 1], F32, tag="pr", name="PR")
                nc.gpsimd.tensor_tensor(PR[:], x[:, 1:T], x[:, 0:T - 1],
                                        op=ALU.mult)
                j4 = p_junk.tile([P, T], F32, tag="junk", name="j4")
                # sum sign(x_t x_{t+1}); crossings = (255 - S)/2
                nc.scalar.activation(j4[:, 0:T - 1], PR[:], AF.Sign,
                                     accum_out=zc_s[:, i2:i2 + 1])

            # ---------- shank z channels ----------
            szv = Xb[:, 8:16, :].rearrange("p (s c) t -> p s c t", s=2)[:, :, 0, :]
            nc.vector.tensor_reduce(out_t[:, b, 2:4], szv, axis=AX.X,
                                    op=ALU.max, apply_absolute_value=True)
            for side, ch in enumerate(SZ):
                nc.vector.bn_stats(bnS_s[:, 2 * b + side, :], Xb[:, ch, :])
                y = Xb[:, ch, :]
                i2 = 2 * b + side
                D = p_scr.tile([P, T - 1], F32, tag="d", name="D")
                nc.gpsimd.tensor_tensor(D[:], y[:, 1:T], y[:, 0:T - 1],
                                        op=ALU.subtract)
                j6 = p_junk.tile([P, T], F32, tag="junk", name="j6")
                nc.scalar.activation(j6[:, 0:T - 1], D[:], AF.Abs,
                                     accum_out=vib_s[:, i2:i2 + 1])

            # ---------- gyro groups ----------
            # peaks: foot+shank groups (start ch 1,5,9,13) in one reduce,
            # thigh groups (16..21) in another
            gfs = Xb[:, 1:17, :].rearrange("p (g c) t -> p g c t", g=4)[:, :, 0:3, :]
            vqb = out_t[:, b, 20:32].rearrange("p (j q) -> p j q", q=4)
            nc.vector.tensor_reduce(vqb[:, 0:2, 2:4], gfs, axis=AX.XY,
                                    op=ALU.max, apply_absolute_value=True)
            gth = Xb[:, 16:22, :].rearrange("p (g c) t -> p g c t", g=2)
            nc.vector.tensor_reduce(vqb[:, 2, 2:4], gth, axis=AX.XY,
                                    op=ALU.max, apply_absolute_value=True)
            for gi, c0 in enumerate((1, 5, 9, 13, 16, 19)):
                ig = 6 * b + gi
                for cc in range(3):
                    nc.vector.bn_stats(bnG_s[:, ig * 3 + cc, :],
                                       Xb[:, c0 + cc, :])

        # ================= final batched scalar phase =================
        def v2(tbl):
            return tbl[:].rearrange("p (b s) -> p b s", s=2)

        def ft(tag, shape=(P, NBLK, 2)):
            return fin.tile(list(shape), F32, tag=tag, name=tag)

        V = nc.vector
        SC = nc.scalar

        # ratio = log1p(f_pk / (s_pk + 1e-4))  [Ln later]
        r_spk = ft("r_spk")
        V.tensor_scalar(r_spk[:], out_t[:, :, 2:4], 1e-4, None, op0=ALU.add)
        V.reciprocal(r_spk[:], r_spk[:])
        ratio_arg = ft("ratio_arg")
        V.tensor_mul(ratio_arg[:], out_t[:, :, 0:2], r_spk[:])

        # total power reciprocal (shared by hf and sc)
        r_tot = ft("r_tot")
        V.tensor_scalar(r_tot[:], v2(tot_s), EPS, None, op0=ALU.add)
        V.reciprocal(r_tot[:], r_tot[:])
        V.tensor_mul(out_t[:, :, 6:8], v2(hf_s), r_tot[:])
        V.scalar_tensor_tensor(out_t[:, :, 16:18], v2(scn_s), 1.0 / NBINS,
                               r_tot[:], op0=ALU.mult, op1=ALU.mult)

        # dur, vib, zcr scaled counts
        V.tensor_scalar(out_t[:, :, 18:20], v2(imp_s), 1.0 / (2 * T), 0.5,
                        op0=ALU.mult, op1=ALU.add)
        V.tensor_scalar(out_t[:, :, 12:14], v2(vib_s), 1.0 / (T - 1), None, op0=ALU.mult)
        V.tensor_scalar(out_t[:, :, 42:44], v2(zc_s), -1.0 / (2 * (T - 1)),
                        0.5, op0=ALU.mult, op1=ALU.add)

        # decay = h0 / (h1 + 128e-6)
        hv = habs_s[:].rearrange("p (b s h) -> p b s h", s=2, h=2)
        dden = ft("dden")
        V.tensor_scalar(dden[:], hv[:, :, :, 1], (T // 2) * EPS, None, op0=ALU.add)
        V.reciprocal(dden[:], dden[:])
        V.tensor_mul(out_t[:, :, 10:12], hv[:, :, :, 0], dden[:])

        # asym_acc = |f_pk - s_pk|
        aa = ft("aa")
        V.tensor_sub(aa[:], out_t[:, :, 0:2], out_t[:, :, 2:4])
        SC.activation(out_t[:, :, 32:34], aa[:], AF.Abs)

        # foot m2 via Parseval:
        # sum x^2 = (2 tot - p0 - pN)/256 with p0 = (256 mu)^2, pN = (256 aN)^2
        #         = tot/128 - 256 (mu^2 + aN^2)
        mv = muN_s[:].rearrange("p (b s h) -> p b s h", s=2, h=2)
        qF = ft("qF")
        V.tensor_mul(qF[:], mv[:, :, :, 0], mv[:, :, :, 0])
        qN = ft("qN")
        V.tensor_mul(qN[:], mv[:, :, :, 1], mv[:, :, :, 1])
        qsum = ft("qsum")
        V.tensor_add(qsum[:], qF[:], qN[:])
        tq = ft("tq")
        V.tensor_scalar(tq[:], v2(tot_s), 1.0 / (T // 2), None, op0=ALU.mult)
        sqF = ft("sqF")           # sum x^2
        V.scalar_tensor_tensor(sqF[:], qsum[:], -float(T), tq[:],
                               op0=ALU.mult, op1=ALU.add)
        m2F = ft("m2F")           # sum (x-mu)^2
        V.scalar_tensor_tensor(m2F[:], qF[:], -float(T), sqF[:],
                               op0=ALU.mult, op1=ALU.add)

        # shank bn-derived: sum y^2 and M2
        meS = bnS_s[:, :, 1].rearrange("p (b s) -> p b s", s=2)
        moS = bnS_s[:, :, 4].rearrange("p (b s) -> p b s", s=2)
        ceS = bnS_s[:, :, 2].rearrange("p (b s) -> p b s", s=2)
        coS = bnS_s[:, :, 5].rearrange("p (b s) -> p b s", s=2)
        sAS = ft("sAS")
        V.tensor_add(sAS[:], ceS, coS)
        uS = ft("uS")
        V.tensor_mul(uS[:], meS, meS)
        vS = ft("vS")
        V.tensor_mul(vS[:], moS, moS)
        wS = ft("wS")
        V.tensor_add(wS[:], uS[:], vS[:])
        sqS = ft("sqS")          # sum y^2
        V.scalar_tensor_tensor(sqS[:], wS[:], float(T // 2), sAS[:],
                               op0=ALU.mult, op1=ALU.add)
        musS = ft("musS")
        V.tensor_add(musS[:], meS, moS)
        qS = ft("qS")
        V.tensor_mul(qS[:], musS[:], musS[:])
        M2S = ft("M2S")          # sum (y - mu)^2
        V.scalar_tensor_tensor(M2S[:], qS[:], -float(T) / 4.0, sqS[:],
                               op0=ALU.mult, op1=ALU.add)

        # var_ratio = log1p(m2F / (M2S + 255e-4))  [Ln later]
        vr = ft("vr")
        V.tensor_scalar(vr[:], M2S[:], (T - 1) * 1e-4, None, op0=ALU.add)
        V.reciprocal(vr[:], vr[:])
        vra = ft("vra")
        V.tensor_mul(vra[:], m2F[:], vr[:])

        # gyro bn-derived per channel -> M2 summed over the 3 channels
        meG = bnG_s[:, :, 1]     # [P, NG*3] strided
        moG = bnG_s[:, :, 4]
        ceG = bnG_s[:, :, 2]
        coG = bnG_s[:, :, 5]
        NG3 = NG * 3
        sAG = ft("sAG", (P, NG3))
        V.tensor_add(sAG[:], ceG, coG)
        uG = ft("uG", (P, NG3))
        V.tensor_mul(uG[:], meG, meG)
        vG = ft("vG", (P, NG3))
        V.tensor_mul(vG[:], moG, moG)
        wG = ft("wG", (P, NG3))
        V.tensor_add(wG[:], uG[:], vG[:])
        sqG = ft("sqG", (P, NG3))
        V.scalar_tensor_tensor(sqG[:], wG[:], float(T // 2), sAG[:],
                               op0=ALU.mult, op1=ALU.add)
        musG = ft("musG", (P, NG3))
        V.tensor_add(musG[:], meG, moG)
        qG = ft("qG", (P, NG3))
        V.tensor_mul(qG[:], musG[:], musG[:])
        M2G = ft("M2G", (P, NG3))
        V.scalar_tensor_tensor(M2G[:], qG[:], -float(T) / 4.0, sqG[:],
                               op0=ALU.mult, op1=ALU.add)
        gM2 = ft("gM2", (P, NG))
        V.reduce_sum(gM2[:], M2G[:].rearrange("p (g c) -> p g c", c=3), axis=AX.X)

        vq = out_t[:, :, 20:32].rearrange("p b (j q) -> p b j q", q=4)

        # ---- Sqrt-set ACT ops ----
        SC.activation(out_t[:, :, 8:10], m2F[:], AF.Sqrt, scale=1.0 / (T - 1))
        rmsF = ft("rmsF")
        SC.activation(rmsF[:], sqF[:], AF.Sqrt, scale=1.0 / T)
        rmsS = ft("rmsS")
        SC.activation(rmsS[:], sqS[:], AF.Sqrt, scale=1.0 / T)

        # kurt/skew (uses std at out[...,8:10])
        sg = ft("sg")
        V.tensor_scalar(sg[:], out_t[:, :, 8:10], 1e-6, None, op0=ALU.max)
        vv = ft("vv")
        V.tensor_mul(vv[:], sg[:], sg[:])
        v4 = ft("v4")
        V.tensor_mul(v4[:], vv[:], vv[:])
        V.reciprocal(v4[:], v4[:])
        kr = ft("kr")
        V.scalar_tensor_tensor(kr[:], v2(m4_s), 1.0 / T, v4[:],
                               op0=ALU.mult, op1=ALU.mult)
        V.tensor_scalar(out_t[:, :, 38:40], kr[:], 30.0, -10.0,
                        op0=ALU.min, op1=ALU.max)
        v3 = ft("v3")
        V.tensor_mul(v3[:], vv[:], sg[:])
        V.reciprocal(v3[:], v3[:])
        sk = ft("sk")
        V.scalar_tensor_tensor(sk[:], v2(m3_s), 1.0 / T, v3[:],
                               op0=ALU.mult, op1=ALU.mult)
        V.tensor_scalar(out_t[:, :, 40:42], sk[:], 10.0, -10.0,
                        op0=ALU.min, op1=ALU.max)

        # trans arg = rmsS / (rmsF + 1e-6)
        rdn = ft("rdn")
        V.tensor_scalar(rdn[:], rmsF[:], EPS, None, op0=ALU.add)
        V.reciprocal(rdn[:], rdn[:])
        targ = ft("targ")
        V.tensor_mul(targ[:], rmsS[:], rdn[:])

        # ---- Ln-set ACT ops (log1p via bias=1) ----
        SC.activation(out_t[:, :, 4:6], ratio_arg[:], AF.Ln, bias=1.0)
        SC.activation(out_t[:, :, 14:16], vra[:], AF.Ln, bias=1.0)
        SC.activation(out_t[:, :, 36:38], targ[:], AF.Ln, bias=1.0)
        SC.activation(vq[:, :, :, 0:2],
                      gM2[:].rearrange("p (b j l) -> p b j l", j=3, l=2),
                      AF.Ln, scale=1.0 / (T - 1), bias=1.0)

        # asym_gy = |fg_var - sg_var| (after log1p)
        ag = ft("ag")
        V.tensor_sub(ag[:], out_t[:, :, 20:22], out_t[:, :, 24:26])
        SC.activation(out_t[:, :, 34:36], ag[:], AF.Abs)

        # ---- store ----
        nc.sync.dma_start(out=out_d.ap().rearrange("(b p) f -> p b f", p=P),
                          in_=out_t[:])


_NC_CACHE = None
_CONSTS = None


def _get_nc():
    global _NC_CACHE, _CONSTS
    if _NC_CACHE is None:
        _NC_CACHE = build_nc()
    if _CONSTS is None:
        _CONSTS = build_consts()
    return _NC_CACHE, _CONSTS


def run(foot, shank, thigh, **kw):
    foot = np.asarray(foot, dtype=np.float32)
    shank = np.asarray(shank, dtype=np.float32)
    thigh = np.asarray(thigh, dtype=np.float32)
    X = np.empty((B_FULL, NCH, T), dtype=np.float32)
    X[:, 0:8] = foot[:, CH_FOOT]
    X[:, 8:16] = shank[:, CH_SHANK]
    X[:, 16:22] = thigh[:, CH_THIGH]

    nc, consts = _get_nc()
    in_maps = []
    for i in range(N_CORES):
        in_maps.append({
            "x": np.ascontiguousarray(X[i * BC:(i + 1) * BC]),
            "w": consts["w"], "wh": consts["wh"], "ident": consts["ident"],
        })
    return run_bass_kernel_spmd(nc, in_maps, core_ids=list(range(N_CORES)),
                                **kw)


def kernel(foot, shank, thigh):
    res = run(foot, shank, thigh)
    return np.concatenate([res.results[i]["out"] for i in range(N_CORES)],
                          axis=0)
